# revision 22
# baseline (speedup 1.0000x reference)
"""Distance-aware multihead attention on 8 Trainium2 NeuronCores.

Problem: B=4, S=1024, D=768, H=12, DK=64, NUM_EMB=10.
  q/k/v = linear projections of query/key/value
  idx[b,i,j] = clip(round(9 * |pos_i - pos_j| / MAXD), 0, 9)
  logits = (q.k^T + qe[b,h,i,idx[b,i,j]]) / 8   where qe = q @ emb_k^T
  out = softmax(logits) @ v

Design (v2):
  - All matmul inputs are bf16, host-pre-transposed so the device does ZERO
    input transposes (the v1 kernel lost 3.3ms to per-element DMA descriptors
    from 4-byte dma transposes).
  - u' = round(dist9) - 0.5 computed once per q-tile (fp32 sqrt + mod-round,
    then bf16 cast which is exact on half-integers). The bias decomposes as
    qe[idx] - qe[0] = sum_{e=1..E} dqe_e * (u' >= e-0.5); per (head,q-tile)
    each term is ONE tensor_scalar op (is_ge + mult with per-partition scalar)
    running in the DVE 4x perf mode on bf16.
  - Terms are merged into the QK PSUM partly via a small DVE add tree and
    partly via identity-matmul accumulation on TensorE (psum += I @ t_e).
  - E is data-adaptive: bands that cannot occur for the given positions are
    dropped at build time (seed-0 data has max idx 8, so E=8).
  - Softmax denominator comes free from a ones-column appended to each V head
    slice in the AV matmul.
  - Activation table discipline: Sqrt lives in its own HW table, so all 4
    sqrts are batched between the Square ops and the Exp ops (2 table loads).

Sharding: core c handles batch c//2, query-half c%2 (512 queries, all heads).
"""
import numpy as np
import ml_dtypes

import concourse.bass as bass
import concourse.tile as tile
from concourse import bacc, mybir
from concourse.bass_utils import run_bass_kernel_spmd
from concourse.masks import make_identity

F32 = mybir.dt.float32
BF16 = mybir.dt.bfloat16
ACT = mybir.ActivationFunctionType
ALU = mybir.AluOpType

B, S, D = 4, 1024, 768
H, DK = 12, 64
NUM_EMB = 10
MAX_DIST = 100000.0 * 2 ** 0.5
SQ = S // 2          # queries per core
NQT = SQ // 128      # q-tiles per core (4)
NKT = S // 128       # k token chunks (8)
NDT = D // 128       # dim tiles (6)
NCORES = 8
SCL9 = 9.0 / MAX_DIST

BF = ml_dtypes.bfloat16


def build_nc(n_e=8, n_dve=3):
    """n_e: number of active bias bands (e = 1..n_e).
    n_dve: how many t_e tiles are merged by a DVE add tree; the rest (and the
    tree root) are accumulated into the QK psum by identity matmuls on PE."""
    nc = bacc.Bacc("TRN2", target_bir_lowering=False, debug=False)

    xqt = nc.dram_tensor("xqt", [D, SQ], BF16, kind="ExternalInput").ap()
    xkt = nc.dram_tensor("xkt", [D, S], BF16, kind="ExternalInput").ap()
    xvt = nc.dram_tensor("xvt", [D, S], BF16, kind="ExternalInput").ap()
    wqt = nc.dram_tensor("wqt", [D, D], BF16, kind="ExternalInput").ap()
    wkt = nc.dram_tensor("wkt", [D, D], BF16, kind="ExternalInput").ap()
    wvt = nc.dram_tensor("wvt", [D, D], BF16, kind="ExternalInput").ap()
    embt = nc.dram_tensor("embt", [DK, NUM_EMB], BF16, kind="ExternalInput").ap()
    bq = nc.dram_tensor("bq", [D], F32, kind="ExternalInput").ap()
    bk = nc.dram_tensor("bk", [D], F32, kind="ExternalInput").ap()
    bv = nc.dram_tensor("bv", [D], F32, kind="ExternalInput").ap()
    pkx = nc.dram_tensor("pkx", [S], F32, kind="ExternalInput").ap()
    pky = nc.dram_tensor("pky", [S], F32, kind="ExternalInput").ap()
    pqx = nc.dram_tensor("pqx", [SQ], F32, kind="ExternalInput").ap()
    pqy = nc.dram_tensor("pqy", [SQ], F32, kind="ExternalInput").ap()
    out = nc.dram_tensor("out", [SQ, D], F32, kind="ExternalOutput").ap()

    with tile.TileContext(nc) as tc:
        with tc.tile_pool(name="persist", bufs=1) as persist:
            # ---- small setup tensors ----
            bq_col = persist.tile([128, NDT], F32)
            bk_col = persist.tile([128, NDT], F32)
            nc.sync.dma_start(out=bq_col[:], in_=bass.AP(tensor=bq.tensor, offset=0, ap=[[1, 128], [128, NDT]]))
            nc.sync.dma_start(out=bk_col[:], in_=bass.AP(tensor=bk.tensor, offset=0, ap=[[1, 128], [128, NDT]]))
            bv_b = persist.tile([128, D], F32)
            nc.sync.dma_start(out=bv_b[:], in_=bass.AP(tensor=bv.tensor, offset=0, ap=[[0, 128], [1, D]]))
            xk_b = persist.tile([128, S], F32)
            yk_b = persist.tile([128, S], F32)
            nc.sync.dma_start(out=xk_b[:], in_=bass.AP(tensor=pkx.tensor, offset=0, ap=[[0, 128], [1, S]]))
            nc.sync.dma_start(out=yk_b[:], in_=bass.AP(tensor=pky.tensor, offset=0, ap=[[0, 128], [1, S]]))
            xq_col = persist.tile([128, NQT], F32)
            yq_col = persist.tile([128, NQT], F32)
            nc.sync.dma_start(out=xq_col[:], in_=bass.AP(tensor=pqx.tensor, offset=0, ap=[[1, 128], [128, NQT]]))
            nc.sync.dma_start(out=yq_col[:], in_=bass.AP(tensor=pqy.tensor, offset=0, ap=[[1, 128], [128, NQT]]))
            # emb^T block-diagonal [128, 20]: rows 0-63 head-even, 64-127 head-odd
            embT_blk = persist.tile([128, 2 * NUM_EMB], BF16)
            nc.vector.memset(embT_blk[:], 0.0)
            nc.sync.dma_start(out=embT_blk[0:64, 0:NUM_EMB], in_=embt[:, :])
            nc.sync.dma_start(out=embT_blk[64:128, NUM_EMB:2 * NUM_EMB], in_=embt[:, :])

            ident = persist.tile([128, 128], BF16)
            make_identity(nc, ident[:])

            # ---- persistent big tensors ----
            kT = persist.tile([128, NDT, S], BF16)        # K^T [dim, token]
            qT = persist.tile([128, NDT, SQ], BF16)       # Q^T [dim, token]
            v_sb = persist.tile([128, NKT, H, DK + 1], BF16)  # V [token, head, dk+1]
            nc.vector.memset(v_sb[:, :, :, DK:DK + 1], 1.0)   # ones col -> denominator
            masks_all = persist.tile([128, NQT, n_e, S], BF16)  # step masks per q-tile
            dqe = persist.tile([128, NQT, H, n_e], F32)   # qe band steps

            # ---- step-mask prep: depends only on positions, so it is issued
            # BEFORE the projections and runs on DVE/Act while PE projects.
            # The pool stays open through the projections (no SBUF reuse
            # serialization).
            # Masks are SIGN masks (+-1): bias = sum_e (dqe_e/2)*sign_e differs
            # from the true bias by a per-row constant, which softmax cancels.
            # xq_col/yq_col hold NEGATED query positions so the subtract folds
            # into the Square's per-partition bias on the scalar engine.
            THRESH2 = [float(((e - 0.5) * MAX_DIST / 9.0) ** 2) for e in range(1, n_e + 1)]
            negT = persist.tile([128, n_e], F32)
            for e in range(n_e):
                nc.vector.memset(negT[:, e:e + 1], -THRESH2[e])
            mprep = tc.tile_pool(name="mprep", bufs=1)
            mp = mprep.__enter__()
            for qt in range(NQT):
                dx2 = mp.tile([128, S], F32, tag="dx2")
                dy2 = mp.tile([128, S], F32, tag="dy2")
                nc.scalar.activation(dx2[:], xk_b[:], ACT.Square, bias=xq_col[:, qt:qt + 1])
                nc.scalar.activation(dy2[:], yk_b[:], ACT.Square, bias=yq_col[:, qt:qt + 1])
                d2 = mp.tile([128, S], F32, tag="d2")
                nc.vector.tensor_add(d2[:], dx2[:], dy2[:])
                for e in range(n_e):
                    nc.scalar.activation(masks_all[:, qt, e, :], d2[:],
                                         ACT.Sign, bias=negT[:, e:e + 1])

            # ---- projections (all bf16; lhsT/rhs host-pre-transposed) ----
            # Order: Q first (qe/dqe depend on it), then K (first heads can
            # start), then V (only needed one head later by the AV stage).
            with tc.tile_pool(name="proj", bufs=1) as pj, \
                 tc.tile_pool(name="pj_ps", bufs=4, space="PSUM") as pj_ps, \
                 tc.tile_pool(name="prep", bufs=1) as prep, \
                 tc.tile_pool(name="qe_ps", bufs=2, space="PSUM") as qe_ps:
                wq_sb = pj.tile([128, NDT, D], BF16)
                xq_sb = pj.tile([128, NDT, SQ], BF16)
                nc.sync.dma_start(out=wq_sb[:], in_=wqt.rearrange("(t p) o -> p t o", p=128))
                nc.sync.dma_start(out=xq_sb[:], in_=xqt.rearrange("(t p) j -> p t j", p=128))
                wk_sb = pj.tile([128, NDT, D], BF16)
                xk_sb = pj.tile([128, NDT, S], BF16)
                nc.sync.dma_start(out=wk_sb[:], in_=wkt.rearrange("(t p) o -> p t o", p=128))
                nc.sync.dma_start(out=xk_sb[:], in_=xkt.rearrange("(t p) j -> p t j", p=128))
                wv_sb = pj.tile([128, NDT, D], BF16)
                xv_sb = pj.tile([128, NDT, S], BF16)
                nc.sync.dma_start(out=wv_sb[:], in_=wvt.rearrange("(t p) o -> p t o", p=128))
                nc.sync.dma_start(out=xv_sb[:], in_=xvt.rearrange("(t p) j -> p t j", p=128))

                for m in range(NDT):
                    ps = pj_ps.tile([128, 512], F32, tag="pj")
                    for t in range(NDT):
                        nc.tensor.matmul(ps[:], wq_sb[:, t, 128 * m:128 * m + 128],
                                         xq_sb[:, t, :],
                                         start=(t == 0), stop=(t == NDT - 1))
                    nc.scalar.activation(qT[:, m, :], ps[:], ACT.Identity,
                                         bias=bq_col[:, m:m + 1])

                # qe -> dqe band steps (only needs qT)
                for qt in range(NQT):
                    qe_psum = qe_ps.tile([128, H * NUM_EMB], F32, tag="qe")
                    for m in range(NDT):
                        nc.tensor.matmul(qe_psum[:, 20 * m:20 * m + 20],
                                         qT[:, m, 128 * qt:128 * qt + 128],
                                         embT_blk[:],
                                         start=True, stop=True)
                    qe_sb = prep.tile([128, H, NUM_EMB], F32, tag="qe_sb")
                    nc.scalar.copy(qe_sb[:], qe_psum[:].rearrange("p (h e) -> p h e", e=NUM_EMB))
                    dq_t = prep.tile([128, H, n_e], F32, tag="dq_t")
                    nc.vector.tensor_tensor(out=dq_t[:],
                                            in0=qe_sb[:, :, 1:1 + n_e],
                                            in1=qe_sb[:, :, 0:n_e], op=ALU.subtract)
                    # halved steps to pair with +-1 sign masks
                    nc.vector.tensor_scalar(out=dqe[:, qt, :, :], in0=dq_t[:],
                                            scalar1=0.5, scalar2=None, op0=ALU.mult)

                for m in range(NDT):
                    for hf in range(2):
                        ps = pj_ps.tile([128, 512], F32, tag="pj")
                        for t in range(NDT):
                            nc.tensor.matmul(ps[:], wk_sb[:, t, 128 * m:128 * m + 128],
                                             xk_sb[:, t, 512 * hf:512 * hf + 512],
                                             start=(t == 0), stop=(t == NDT - 1))
                        nc.scalar.activation(kT[:, m, 512 * hf:512 * hf + 512], ps[:],
                                             ACT.Identity, bias=bk_col[:, m:m + 1])

                for m in range(NKT):
                    for hf in range(2):
                        ps = pj_ps.tile([128, 384], F32, tag="pj")
                        for t in range(NDT):
                            nc.tensor.matmul(ps[:], xv_sb[:, t, 128 * m:128 * m + 128],
                                             wv_sb[:, t, 384 * hf:384 * hf + 384],
                                             start=(t == 0), stop=(t == NDT - 1))
                        # scatter 6 head slices [128, 6, 64] -> v_sb[:, m, 6hf:6hf+6, 0:64]
                        nc.scalar.copy(
                            v_sb[:, m, 6 * hf:6 * hf + 6, 0:DK],
                            ps[:].rearrange("p (h d) -> p h d", d=DK))

            mprep.__exit__(None, None, None)

            # ---- attention: software-pipelined over (qt, h) ----
            with tc.tile_pool(name="att", bufs=2) as att, \
                 tc.tile_pool(name="osb", bufs=2) as osb, \
                 tc.tile_pool(name="qk_ps", bufs=2, space="PSUM") as qk_ps, \
                 tc.tile_pool(name="pt_ps", bufs=2, space="PSUM") as pt_ps, \
                 tc.tile_pool(name="av_ps", bufs=2, space="PSUM") as av_ps:

                prev = None          # (qt, h, p_sb, o_tile)
                pend = None          # (qt, h, av, o_tile) awaiting normalize
                o_tile = None

                def finish_pe(prev):
                    """transpose P (PE), copy to SBUF (Act), AV matmuls (PE)."""
                    qt_p, h_p, p_sb, o_t = prev
                    ptp = pt_ps.tile([128, NKT, 128], BF16, tag="ptp")
                    for c in range(NKT):
                        nc.tensor.transpose(ptp[:, c, :], p_sb[:, 128 * c:128 * c + 128], ident[:])
                    pT = att.tile([128, NKT, 128], BF16, tag="pT")
                    nc.scalar.copy(pT[:], ptp[:])
                    av = av_ps.tile([128, DK + 1], F32, tag="av")
                    for c in range(NKT):
                        nc.tensor.matmul(av[:], pT[:, c, :], v_sb[:, c, h_p, :],
                                         start=(c == 0), stop=(c == NKT - 1))
                    return (qt_p, h_p, av, o_t)

                def finish_post(pend):
                    """normalize by the ones-column row sum, add bv, store."""
                    qt_p, h_p, av, o_t = pend
                    recip = att.tile([128, 1], F32, tag="recip")
                    nc.vector.reciprocal(recip[:], av[:, DK:DK + 1])
                    nc.vector.scalar_tensor_tensor(
                        out=o_t[:, h_p, :], in0=av[:, 0:DK], scalar=recip[:],
                        in1=bv_b[:, DK * h_p:DK * h_p + DK], op0=ALU.mult, op1=ALU.add)
                    if h_p == H - 1:
                        nc.sync.dma_start(
                            out=out[128 * qt_p:128 * qt_p + 128, :],
                            in_=o_t[:].rearrange("p h d -> p (h d)"))

                for qt in range(NQT):
                    o_tile = osb.tile([128, H, DK], F32, tag="o")
                    for h in range(H):
                        off = (64 * h) % 128
                        # PE leads with prev head's transpose+AV (ready work)
                        if prev is not None:
                            pend = finish_pe(prev)
                        # --- bias band tiles: t_e = sign_e * (dqe_e/2), bf16 4x ---
                        tt = att.tile([128, n_e, S], BF16, tag="tt")
                        for e in range(n_e):
                            nc.vector.tensor_scalar(
                                out=tt[:, e, :], in0=masks_all[:, qt, e, :],
                                scalar1=dqe[:, qt, h, e:e + 1], scalar2=None,
                                op0=ALU.mult)
                        # --- small DVE tree over the first n_dve tiles ---
                        # (GpSimd adds were tried here: they contend with DVE
                        # for SBUF ports and slowed every DVE op ~50%.)
                        r = tt[:, 0, :]
                        for d in range(1, n_dve):
                            racc = att.tile([128, S], BF16, tag=f"racc{d % 2}")
                            nc.vector.tensor_tensor(out=racc[:], in0=r, in1=tt[:, d, :], op=ALU.add)
                            r = racc[:]
                        inj = [tt[:, e, :] for e in range(n_dve, n_e)] + [r]
                        # --- qk + injected bias accumulation in PSUM ---
                        qk = qk_ps.tile([128, S], F32, tag="qk")
                        for hf in range(2):
                            sl = slice(512 * hf, 512 * hf + 512)
                            nc.tensor.matmul(qk[:, sl],
                                             qT[off:off + 64, h // 2, 128 * qt:128 * qt + 128],
                                             kT[off:off + 64, h // 2, sl],
                                             start=True, stop=False)
                            for ii, tsl in enumerate(inj):
                                nc.tensor.matmul(qk[:, sl], ident[:], tsl[:, sl],
                                                 start=False, stop=(ii == len(inj) - 1))
                        # --- P = exp(logits/8) ---
                        p_sb = att.tile([128, S], BF16, tag="p")
                        nc.scalar.activation(p_sb[:], qk[:], ACT.Exp, scale=0.125)
                        if pend is not None:
                            finish_post(pend)
                            pend = None
                        prev = (qt, h, p_sb, o_tile)
                # drain
                finish_post(finish_pe(prev))
    nc.compile()
    return nc


_NC_CACHE = {}


def _get_nc(n_e=None):
    if n_e is None:
        n_e = _NC_CACHE.get("last", 8)
    if n_e not in _NC_CACHE:
        _NC_CACHE[n_e] = build_nc(n_e=n_e)
    _NC_CACHE["last"] = n_e
    return _NC_CACHE[n_e]


def _make_in_maps(inputs):
    query = np.asarray(inputs["query"], dtype=np.float32)
    key = np.asarray(inputs["key"], dtype=np.float32)
    value = np.asarray(inputs["value"], dtype=np.float32)
    tp = np.asarray(inputs["tile_positions"], dtype=np.float32)
    Wq = np.asarray(inputs["Wq"], dtype=np.float32)
    Wk = np.asarray(inputs["Wk"], dtype=np.float32)
    Wv = np.asarray(inputs["Wv"], dtype=np.float32)
    bq = np.asarray(inputs["bq"], dtype=np.float32)
    bk = np.asarray(inputs["bk"], dtype=np.float32)
    bv = np.asarray(inputs["bv"], dtype=np.float32)
    emb = np.asarray(inputs["emb_k"], dtype=np.float32)

    wqt = np.ascontiguousarray(Wq.T.astype(BF))
    wkt = np.ascontiguousarray(Wk.T.astype(BF))
    wvt = np.ascontiguousarray(Wv.T.astype(BF))
    embt = np.ascontiguousarray(emb.T.astype(BF))

    in_maps = []
    for c in range(NCORES):
        b, qh = c // 2, c % 2
        sl = slice(qh * SQ, (qh + 1) * SQ)
        in_maps.append({
            "xqt": np.ascontiguousarray(query[b, sl].T.astype(BF)),
            "xkt": np.ascontiguousarray(key[b].T.astype(BF)),
            "xvt": np.ascontiguousarray(value[b].T.astype(BF)),
            "wqt": wqt, "wkt": wkt, "wvt": wvt, "embt": embt,
            "bq": bq, "bk": bk, "bv": bv,
            "pkx": np.ascontiguousarray(tp[b, :, 0]),
            "pky": np.ascontiguousarray(tp[b, :, 1]),
            # negated: folded into the Square activation's per-partition bias
            "pqx": np.ascontiguousarray(-tp[b, sl, 0]),
            "pqy": np.ascontiguousarray(-tp[b, sl, 1]),
        })
    return in_maps


def _active_bands(tp):
    """Highest band index that actually occurs for these positions."""
    mx = 0.0
    for b in range(tp.shape[0]):
        p = tp[b]
        d2 = ((p[:, None, :] - p[None, :, :]) ** 2).sum(-1)
        mx = max(mx, float(d2.max()))
    max_idx = int(np.floor(9.0 * np.sqrt(mx) / MAX_DIST + 0.5))
    return max(1, min(max_idx, NUM_EMB - 1))


def kernel(query, key, value, tile_positions, Wq, bq, Wk, bk, Wv, bv, emb_k):
    inputs = {"query": query, "key": key, "value": value,
              "tile_positions": tile_positions,
              "Wq": Wq, "bq": bq, "Wk": Wk, "bk": bk, "Wv": Wv, "bv": bv,
              "emb_k": emb_k}
    tp = np.asarray(tile_positions, dtype=np.float32)
    n_e = _active_bands(tp)
    nc = _get_nc(n_e)
    in_maps = _make_in_maps(inputs)
    res = run_bass_kernel_spmd(nc, in_maps, core_ids=list(range(NCORES)))
    out = np.empty((B, S, D), np.float32)
    for c in range(NCORES):
        b, qh = c // 2, c % 2
        out[b, qh * SQ:(qh + 1) * SQ] = res.results[c]["out"]
    return out


# revision 26
# speedup vs baseline: 1.0900x; 1.0900x over previous
"""Distance-aware multihead attention on 8 Trainium2 NeuronCores.

Problem: B=4, S=1024, D=768, H=12, DK=64, NUM_EMB=10.
  q/k/v = linear projections of query/key/value
  idx[b,i,j] = clip(round(9 * |pos_i - pos_j| / MAXD), 0, 9)
  logits = (q.k^T + qe[b,h,i,idx[b,i,j]]) / 8   where qe = q @ emb_k^T
  out = softmax(logits) @ v

Design (v2):
  - All matmul inputs are bf16, host-pre-transposed so the device does ZERO
    input transposes (the v1 kernel lost 3.3ms to per-element DMA descriptors
    from 4-byte dma transposes).
  - u' = round(dist9) - 0.5 computed once per q-tile (fp32 sqrt + mod-round,
    then bf16 cast which is exact on half-integers). The bias decomposes as
    qe[idx] - qe[0] = sum_{e=1..E} dqe_e * (u' >= e-0.5); per (head,q-tile)
    each term is ONE tensor_scalar op (is_ge + mult with per-partition scalar)
    running in the DVE 4x perf mode on bf16.
  - Terms are merged into the QK PSUM partly via a small DVE add tree and
    partly via identity-matmul accumulation on TensorE (psum += I @ t_e).
  - E is data-adaptive: bands that cannot occur for the given positions are
    dropped at build time (seed-0 data has max idx 8, so E=8).
  - Softmax denominator comes free from a ones-column appended to each V head
    slice in the AV matmul.
  - Activation table discipline: Sqrt lives in its own HW table, so all 4
    sqrts are batched between the Square ops and the Exp ops (2 table loads).

Sharding: core c handles batch c//2, query-half c%2 (512 queries, all heads).
"""
import numpy as np
import ml_dtypes

import concourse.bass as bass
import concourse.tile as tile
from concourse import bacc, mybir
from concourse.bass_utils import run_bass_kernel_spmd
from concourse.masks import make_identity

F32 = mybir.dt.float32
BF16 = mybir.dt.bfloat16
ACT = mybir.ActivationFunctionType
ALU = mybir.AluOpType

B, S, D = 4, 1024, 768
H, DK = 12, 64
NUM_EMB = 10
MAX_DIST = 100000.0 * 2 ** 0.5
SQ = S // 2          # queries per core
NQT = SQ // 128      # q-tiles per core (4)
NKT = S // 128       # k token chunks (8)
NDT = D // 128       # dim tiles (6)
NCORES = 8
SCL9 = 9.0 / MAX_DIST

BF = ml_dtypes.bfloat16


def build_nc(n_e=8, n_dve=3):
    """n_e: number of active bias bands (e = 1..n_e).
    n_dve: how many t_e tiles are merged by a DVE add tree; the rest (and the
    tree root) are accumulated into the QK psum by identity matmuls on PE."""
    nc = bacc.Bacc("TRN2", target_bir_lowering=False, debug=False)

    xqt = nc.dram_tensor("xqt", [D, SQ], BF16, kind="ExternalInput").ap()
    xkt = nc.dram_tensor("xkt", [D, S], BF16, kind="ExternalInput").ap()
    xvt = nc.dram_tensor("xvt", [D, S], BF16, kind="ExternalInput").ap()
    wqt = nc.dram_tensor("wqt", [D, D], BF16, kind="ExternalInput").ap()
    wkt = nc.dram_tensor("wkt", [D, D], BF16, kind="ExternalInput").ap()
    wvt = nc.dram_tensor("wvt", [D, D], BF16, kind="ExternalInput").ap()
    embt = nc.dram_tensor("embt", [DK, NUM_EMB], BF16, kind="ExternalInput").ap()
    bq = nc.dram_tensor("bq", [D], F32, kind="ExternalInput").ap()
    bk = nc.dram_tensor("bk", [D], F32, kind="ExternalInput").ap()
    bv = nc.dram_tensor("bv", [D], F32, kind="ExternalInput").ap()
    pkx = nc.dram_tensor("pkx", [S], F32, kind="ExternalInput").ap()
    pky = nc.dram_tensor("pky", [S], F32, kind="ExternalInput").ap()
    pqx = nc.dram_tensor("pqx", [SQ], F32, kind="ExternalInput").ap()
    pqy = nc.dram_tensor("pqy", [SQ], F32, kind="ExternalInput").ap()
    out = nc.dram_tensor("out", [SQ, D], F32, kind="ExternalOutput").ap()

    with tile.TileContext(nc) as tc:
        with tc.tile_pool(name="persist", bufs=1) as persist:
            # ---- small setup tensors ----
            bq_col = persist.tile([128, NDT], F32)
            bk_col = persist.tile([128, NDT], F32)
            nc.sync.dma_start(out=bq_col[:], in_=bass.AP(tensor=bq.tensor, offset=0, ap=[[1, 128], [128, NDT]]))
            nc.sync.dma_start(out=bk_col[:], in_=bass.AP(tensor=bk.tensor, offset=0, ap=[[1, 128], [128, NDT]]))
            bv_b = persist.tile([128, D], F32)
            nc.sync.dma_start(out=bv_b[:], in_=bass.AP(tensor=bv.tensor, offset=0, ap=[[0, 128], [1, D]]))
            xk_b = persist.tile([128, S], F32)
            yk_b = persist.tile([128, S], F32)
            nc.sync.dma_start(out=xk_b[:], in_=bass.AP(tensor=pkx.tensor, offset=0, ap=[[0, 128], [1, S]]))
            nc.sync.dma_start(out=yk_b[:], in_=bass.AP(tensor=pky.tensor, offset=0, ap=[[0, 128], [1, S]]))
            xq_col = persist.tile([128, NQT], F32)
            yq_col = persist.tile([128, NQT], F32)
            nc.sync.dma_start(out=xq_col[:], in_=bass.AP(tensor=pqx.tensor, offset=0, ap=[[1, 128], [128, NQT]]))
            nc.sync.dma_start(out=yq_col[:], in_=bass.AP(tensor=pqy.tensor, offset=0, ap=[[1, 128], [128, NQT]]))
            # emb^T block-diagonal [128, 20]: rows 0-63 head-even, 64-127 head-odd
            embT_blk = persist.tile([128, 2 * NUM_EMB], BF16)
            nc.vector.memset(embT_blk[:], 0.0)
            nc.sync.dma_start(out=embT_blk[0:64, 0:NUM_EMB], in_=embt[:, :])
            nc.sync.dma_start(out=embT_blk[64:128, NUM_EMB:2 * NUM_EMB], in_=embt[:, :])

            ident = persist.tile([128, 128], BF16)
            make_identity(nc, ident[:])

            # ---- persistent big tensors ----
            kT = persist.tile([128, NDT, S], BF16)        # K^T [dim, token]
            qT = persist.tile([128, NDT, SQ], BF16)       # Q^T [dim, token]
            v_sb = persist.tile([128, NKT, H, DK + 1], BF16)  # V [token, head, dk+1]
            nc.vector.memset(v_sb[:, :, :, DK:DK + 1], 1.0)   # ones col -> denominator
            masks_all = persist.tile([128, NQT, n_e, S], BF16)  # step masks per q-tile
            dqe = persist.tile([128, NQT, H, n_e], F32)   # qe band steps

            # ---- step-mask prep: depends only on positions, so it is issued
            # BEFORE the projections and runs on DVE (+2 Act squares) while PE
            # projects and Act handles the projection copies. The pool stays
            # open through the projections (no SBUF-reuse serialization).
            # (Sign-masks on Act were tried: they stall the in-order Act queue
            # ahead of the projection copies and lose ~35us.)
            THRESH2 = [float(((e - 0.5) * MAX_DIST / 9.0) ** 2) for e in range(1, n_e + 1)]
            mprep = tc.tile_pool(name="mprep", bufs=1)
            mp = mprep.__enter__()
            for qt in range(NQT):
                dx = mp.tile([128, S], F32, tag="dx")
                dy = mp.tile([128, S], F32, tag="dy")
                nc.vector.tensor_scalar(out=dx[:], in0=xk_b[:], scalar1=xq_col[:, qt:qt + 1],
                                        scalar2=None, op0=ALU.subtract)
                nc.vector.tensor_scalar(out=dy[:], in0=yk_b[:], scalar1=yq_col[:, qt:qt + 1],
                                        scalar2=None, op0=ALU.subtract)
                dx2 = mp.tile([128, S], F32, tag="dx2")
                dy2 = mp.tile([128, S], F32, tag="dy2")
                nc.scalar.activation(dx2[:], dx[:], ACT.Square)
                nc.scalar.activation(dy2[:], dy[:], ACT.Square)
                d2 = mp.tile([128, S], F32, tag="d2")
                nc.vector.tensor_add(d2[:], dx2[:], dy2[:])
                for e in range(n_e):
                    nc.vector.tensor_scalar(out=masks_all[:, qt, e, :], in0=d2[:],
                                            scalar1=THRESH2[e], scalar2=None,
                                            op0=ALU.is_ge)

            # ---- projections (all bf16; lhsT/rhs host-pre-transposed) ----
            # Order: Q first (qe/dqe depend on it), then K (first heads can
            # start), then V (only needed one head later by the AV stage).
            with tc.tile_pool(name="proj", bufs=1) as pj, \
                 tc.tile_pool(name="pj_ps", bufs=4, space="PSUM") as pj_ps, \
                 tc.tile_pool(name="prep", bufs=1) as prep, \
                 tc.tile_pool(name="qe_ps", bufs=2, space="PSUM") as qe_ps:
                wq_sb = pj.tile([128, NDT, D], BF16)
                xq_sb = pj.tile([128, NDT, SQ], BF16)
                nc.sync.dma_start(out=wq_sb[:], in_=wqt.rearrange("(t p) o -> p t o", p=128))
                nc.sync.dma_start(out=xq_sb[:], in_=xqt.rearrange("(t p) j -> p t j", p=128))
                wk_sb = pj.tile([128, NDT, D], BF16)
                xk_sb = pj.tile([128, NDT, S], BF16)
                nc.sync.dma_start(out=wk_sb[:], in_=wkt.rearrange("(t p) o -> p t o", p=128))
                nc.sync.dma_start(out=xk_sb[:], in_=xkt.rearrange("(t p) j -> p t j", p=128))
                wv_sb = pj.tile([128, NDT, D], BF16)
                xv_sb = pj.tile([128, NDT, S], BF16)
                nc.sync.dma_start(out=wv_sb[:], in_=wvt.rearrange("(t p) o -> p t o", p=128))
                nc.sync.dma_start(out=xv_sb[:], in_=xvt.rearrange("(t p) j -> p t j", p=128))

                for m in range(NDT):
                    ps = pj_ps.tile([128, 512], F32, tag="pj")
                    for t in range(NDT):
                        nc.tensor.matmul(ps[:], wq_sb[:, t, 128 * m:128 * m + 128],
                                         xq_sb[:, t, :],
                                         start=(t == 0), stop=(t == NDT - 1))
                    nc.scalar.activation(qT[:, m, :], ps[:], ACT.Identity,
                                         bias=bq_col[:, m:m + 1])

                # qe -> dqe band steps (only needs qT)
                for qt in range(NQT):
                    qe_psum = qe_ps.tile([128, H * NUM_EMB], F32, tag="qe")
                    for m in range(NDT):
                        nc.tensor.matmul(qe_psum[:, 20 * m:20 * m + 20],
                                         qT[:, m, 128 * qt:128 * qt + 128],
                                         embT_blk[:],
                                         start=True, stop=True)
                    qe_sb = prep.tile([128, H, NUM_EMB], F32, tag="qe_sb")
                    nc.scalar.copy(qe_sb[:], qe_psum[:].rearrange("p (h e) -> p h e", e=NUM_EMB))
                    nc.vector.tensor_tensor(out=dqe[:, qt, :, :],
                                            in0=qe_sb[:, :, 1:1 + n_e],
                                            in1=qe_sb[:, :, 0:n_e], op=ALU.subtract)

                for m in range(NDT):
                    for hf in range(2):
                        ps = pj_ps.tile([128, 512], F32, tag="pj")
                        for t in range(NDT):
                            nc.tensor.matmul(ps[:], wk_sb[:, t, 128 * m:128 * m + 128],
                                             xk_sb[:, t, 512 * hf:512 * hf + 512],
                                             start=(t == 0), stop=(t == NDT - 1))
                        nc.scalar.activation(kT[:, m, 512 * hf:512 * hf + 512], ps[:],
                                             ACT.Identity, bias=bk_col[:, m:m + 1])

                for m in range(NKT):
                    for hf in range(2):
                        ps = pj_ps.tile([128, 384], F32, tag="pj")
                        for t in range(NDT):
                            nc.tensor.matmul(ps[:], xv_sb[:, t, 128 * m:128 * m + 128],
                                             wv_sb[:, t, 384 * hf:384 * hf + 384],
                                             start=(t == 0), stop=(t == NDT - 1))
                        # scatter 6 head slices [128, 6, 64] -> v_sb[:, m, 6hf:6hf+6, 0:64]
                        nc.scalar.copy(
                            v_sb[:, m, 6 * hf:6 * hf + 6, 0:DK],
                            ps[:].rearrange("p (h d) -> p h d", d=DK))

            mprep.__exit__(None, None, None)

            # ---- attention: software-pipelined over (qt, h) ----
            with tc.tile_pool(name="att", bufs=2) as att, \
                 tc.tile_pool(name="osb", bufs=2) as osb, \
                 tc.tile_pool(name="qk_ps", bufs=2, space="PSUM") as qk_ps, \
                 tc.tile_pool(name="pt_ps", bufs=2, space="PSUM") as pt_ps, \
                 tc.tile_pool(name="av_ps", bufs=2, space="PSUM") as av_ps:

                prev = None          # (qt, h, p_sb, o_tile)
                pend = None          # (qt, h, av, o_tile) awaiting normalize
                o_tile = None

                def finish_pe(prev):
                    """transpose P (PE), copy to SBUF (Act), AV matmuls (PE)."""
                    qt_p, h_p, p_sb, o_t = prev
                    ptp = pt_ps.tile([128, NKT, 128], BF16, tag="ptp")
                    for c in range(NKT):
                        nc.tensor.transpose(ptp[:, c, :], p_sb[:, 128 * c:128 * c + 128], ident[:])
                    pT = att.tile([128, NKT, 128], BF16, tag="pT")
                    nc.scalar.copy(pT[:], ptp[:])
                    av = av_ps.tile([128, DK + 1], F32, tag="av")
                    for c in range(NKT):
                        nc.tensor.matmul(av[:], pT[:, c, :], v_sb[:, c, h_p, :],
                                         start=(c == 0), stop=(c == NKT - 1))
                    return (qt_p, h_p, av, o_t)

                def finish_post(pend):
                    """normalize by the ones-column row sum, add bv, store."""
                    qt_p, h_p, av, o_t = pend
                    recip = att.tile([128, 1], F32, tag="recip")
                    nc.vector.reciprocal(recip[:], av[:, DK:DK + 1])
                    nc.vector.scalar_tensor_tensor(
                        out=o_t[:, h_p, :], in0=av[:, 0:DK], scalar=recip[:],
                        in1=bv_b[:, DK * h_p:DK * h_p + DK], op0=ALU.mult, op1=ALU.add)
                    if h_p == H - 1:
                        nc.sync.dma_start(
                            out=out[128 * qt_p:128 * qt_p + 128, :],
                            in_=o_t[:].rearrange("p h d -> p (h d)"))

                for qt in range(NQT):
                    o_tile = osb.tile([128, H, DK], F32, tag="o")
                    for h in range(H):
                        off = (64 * h) % 128
                        # PE leads with prev head's transpose+AV (ready work)
                        if prev is not None:
                            pend = finish_pe(prev)
                        # --- bias band tiles: t_e = mask_e * dqe_e, bf16 4x
                        # (bufs=3: lets DVE run a head ahead of the injects) ---
                        tt = att.tile([128, n_e, S], BF16, tag="tt", bufs=3)
                        for e in range(n_e):
                            nc.vector.tensor_scalar(
                                out=tt[:, e, :], in0=masks_all[:, qt, e, :],
                                scalar1=dqe[:, qt, h, e:e + 1], scalar2=None,
                                op0=ALU.mult)
                        # --- small DVE tree over the first n_dve tiles ---
                        # (GpSimd adds were tried here: they contend with DVE
                        # for SBUF ports and slowed every DVE op ~50%.)
                        r = tt[:, 0, :]
                        for d in range(1, n_dve):
                            racc = att.tile([128, S], BF16, tag=f"racc{d % 2}")
                            nc.vector.tensor_tensor(out=racc[:], in0=r, in1=tt[:, d, :], op=ALU.add)
                            r = racc[:]
                        inj = [tt[:, e, :] for e in range(n_dve, n_e)] + [r]
                        # --- qk + injected bias accumulation in PSUM ---
                        qk = qk_ps.tile([128, S], F32, tag="qk")
                        for hf in range(2):
                            sl = slice(512 * hf, 512 * hf + 512)
                            nc.tensor.matmul(qk[:, sl],
                                             qT[off:off + 64, h // 2, 128 * qt:128 * qt + 128],
                                             kT[off:off + 64, h // 2, sl],
                                             start=True, stop=False)
                            for ii, tsl in enumerate(inj):
                                nc.tensor.matmul(qk[:, sl], ident[:], tsl[:, sl],
                                                 start=False, stop=(ii == len(inj) - 1))
                        # --- P = exp(logits/8) ---
                        p_sb = att.tile([128, S], BF16, tag="p")
                        nc.scalar.activation(p_sb[:], qk[:], ACT.Exp, scale=0.125)
                        if pend is not None:
                            finish_post(pend)
                            pend = None
                        prev = (qt, h, p_sb, o_tile)
                # drain
                finish_post(finish_pe(prev))
    nc.compile()
    return nc


_NC_CACHE = {}


def _get_nc(n_e=None):
    if n_e is None:
        n_e = _NC_CACHE.get("last", 8)
    if n_e not in _NC_CACHE:
        _NC_CACHE[n_e] = build_nc(n_e=n_e)
    _NC_CACHE["last"] = n_e
    return _NC_CACHE[n_e]


def _make_in_maps(inputs):
    query = np.asarray(inputs["query"], dtype=np.float32)
    key = np.asarray(inputs["key"], dtype=np.float32)
    value = np.asarray(inputs["value"], dtype=np.float32)
    tp = np.asarray(inputs["tile_positions"], dtype=np.float32)
    Wq = np.asarray(inputs["Wq"], dtype=np.float32)
    Wk = np.asarray(inputs["Wk"], dtype=np.float32)
    Wv = np.asarray(inputs["Wv"], dtype=np.float32)
    bq = np.asarray(inputs["bq"], dtype=np.float32)
    bk = np.asarray(inputs["bk"], dtype=np.float32)
    bv = np.asarray(inputs["bv"], dtype=np.float32)
    emb = np.asarray(inputs["emb_k"], dtype=np.float32)

    wqt = np.ascontiguousarray(Wq.T.astype(BF))
    wkt = np.ascontiguousarray(Wk.T.astype(BF))
    wvt = np.ascontiguousarray(Wv.T.astype(BF))
    embt = np.ascontiguousarray(emb.T.astype(BF))

    in_maps = []
    for c in range(NCORES):
        b, qh = c // 2, c % 2
        sl = slice(qh * SQ, (qh + 1) * SQ)
        in_maps.append({
            "xqt": np.ascontiguousarray(query[b, sl].T.astype(BF)),
            "xkt": np.ascontiguousarray(key[b].T.astype(BF)),
            "xvt": np.ascontiguousarray(value[b].T.astype(BF)),
            "wqt": wqt, "wkt": wkt, "wvt": wvt, "embt": embt,
            "bq": bq, "bk": bk, "bv": bv,
            "pkx": np.ascontiguousarray(tp[b, :, 0]),
            "pky": np.ascontiguousarray(tp[b, :, 1]),
            "pqx": np.ascontiguousarray(tp[b, sl, 0]),
            "pqy": np.ascontiguousarray(tp[b, sl, 1]),
        })
    return in_maps


def _active_bands(tp):
    """Highest band index that actually occurs for these positions."""
    mx = 0.0
    for b in range(tp.shape[0]):
        p = tp[b]
        d2 = ((p[:, None, :] - p[None, :, :]) ** 2).sum(-1)
        mx = max(mx, float(d2.max()))
    max_idx = int(np.floor(9.0 * np.sqrt(mx) / MAX_DIST + 0.5))
    return max(1, min(max_idx, NUM_EMB - 1))


def kernel(query, key, value, tile_positions, Wq, bq, Wk, bk, Wv, bv, emb_k):
    inputs = {"query": query, "key": key, "value": value,
              "tile_positions": tile_positions,
              "Wq": Wq, "bq": bq, "Wk": Wk, "bk": bk, "Wv": Wv, "bv": bv,
              "emb_k": emb_k}
    tp = np.asarray(tile_positions, dtype=np.float32)
    n_e = _active_bands(tp)
    nc = _get_nc(n_e)
    in_maps = _make_in_maps(inputs)
    res = run_bass_kernel_spmd(nc, in_maps, core_ids=list(range(NCORES)))
    out = np.empty((B, S, D), np.float32)
    for c in range(NCORES):
        b, qh = c // 2, c % 2
        out[b, qh * SQ:(qh + 1) * SQ] = res.results[c]["out"]
    return out


# revision 28
# speedup vs baseline: 1.0937x; 1.0034x over previous
"""Distance-aware multihead attention on 8 Trainium2 NeuronCores.

Problem: B=4, S=1024, D=768, H=12, DK=64, NUM_EMB=10.
  q/k/v = linear projections of query/key/value
  idx[b,i,j] = clip(round(9 * |pos_i - pos_j| / MAXD), 0, 9)
  logits = (q.k^T + qe[b,h,i,idx[b,i,j]]) / 8   where qe = q @ emb_k^T
  out = softmax(logits) @ v

Design:
  - All matmul inputs are bf16, host-pre-transposed so the device does ZERO
    input transposes (the baseline lost ~3.1ms of 3.5ms to per-element DMA
    descriptors from 4-byte dma transposes).
  - Step masks (d2 >= T_e^2, fp32-exact compare, bf16 0/1 output) are built
    once per q-tile on DVE, issued BEFORE the projections so they overlap the
    PE-bound startup. The bias decomposes as qe[idx] - qe[0] =
    sum_{e=1..E} dqe_e * mask_e; per (head, q-tile) each band term is ONE
    tensor_scalar mult with a per-partition dqe scalar, running in the DVE 4x
    perf mode on bf16 (~395ns/tile; scalar_tensor_tensor has NO fast mode,
    which is why the baseline's STT chain was 6x slower).
  - Band tiles are merged into the QK PSUM partly via a small DVE add tree
    (n_dve) and partly via identity-matmul accumulation on TensorE
    (psum += I @ t_e, ~170ns per [128,512] half).
  - E is data-adaptive: bands that cannot occur for the given positions are
    dropped at build time (seed-0 data has max idx 8, so E=8).
  - Softmax denominator comes free from a ones-column appended to each V head
    slice in the AV matmul; P^T comes from PE transposes with the PSUM->SBUF
    copy on the scalar engine.
  - The (qt, h) loop is software-pipelined: PE leads each iteration with the
    previous head's transpose+AV so it never stalls on the current head's
    DVE band tiles.
  - Engines measured ~: DVE 85%, PE 90%, Act 43% in steady state.
  - Rejected variants (measured slower): GpSimd tree adds (SBUF port
    contention slows all DVE ops ~50%); Sign-masks / band-mults on the
    scalar engine (stalls the in-order Act queue ahead of projection copies
    and the exp chain).

Sharding: core c handles batch c//2, query-half c%2 (512 queries, all heads).
"""
import numpy as np
import ml_dtypes

import concourse.bass as bass
import concourse.tile as tile
from concourse import bacc, mybir
from concourse.bass_utils import run_bass_kernel_spmd
from concourse.masks import make_identity

F32 = mybir.dt.float32
BF16 = mybir.dt.bfloat16
ACT = mybir.ActivationFunctionType
ALU = mybir.AluOpType

B, S, D = 4, 1024, 768
H, DK = 12, 64
NUM_EMB = 10
MAX_DIST = 100000.0 * 2 ** 0.5
SQ = S // 2          # queries per core
NQT = SQ // 128      # q-tiles per core (4)
NKT = S // 128       # k token chunks (8)
NDT = D // 128       # dim tiles (6)
NCORES = 8
SCL9 = 9.0 / MAX_DIST

BF = ml_dtypes.bfloat16


def build_nc(n_e=8, n_dve=3):
    """n_e: number of active bias bands (e = 1..n_e).
    n_dve: how many t_e tiles are merged by a DVE add tree; the rest (and the
    tree root) are accumulated into the QK psum by identity matmuls on PE."""
    nc = bacc.Bacc("TRN2", target_bir_lowering=False, debug=False)

    xqt = nc.dram_tensor("xqt", [D, SQ], BF16, kind="ExternalInput").ap()
    xkt = nc.dram_tensor("xkt", [D, S], BF16, kind="ExternalInput").ap()
    xvt = nc.dram_tensor("xvt", [D, S], BF16, kind="ExternalInput").ap()
    wqt = nc.dram_tensor("wqt", [D, D], BF16, kind="ExternalInput").ap()
    wkt = nc.dram_tensor("wkt", [D, D], BF16, kind="ExternalInput").ap()
    wvt = nc.dram_tensor("wvt", [D, D], BF16, kind="ExternalInput").ap()
    embt = nc.dram_tensor("embt", [DK, NUM_EMB], BF16, kind="ExternalInput").ap()
    bq = nc.dram_tensor("bq", [D], F32, kind="ExternalInput").ap()
    bk = nc.dram_tensor("bk", [D], F32, kind="ExternalInput").ap()
    bv = nc.dram_tensor("bv", [D], F32, kind="ExternalInput").ap()
    pkx = nc.dram_tensor("pkx", [S], F32, kind="ExternalInput").ap()
    pky = nc.dram_tensor("pky", [S], F32, kind="ExternalInput").ap()
    pqx = nc.dram_tensor("pqx", [SQ], F32, kind="ExternalInput").ap()
    pqy = nc.dram_tensor("pqy", [SQ], F32, kind="ExternalInput").ap()
    out = nc.dram_tensor("out", [SQ, D], F32, kind="ExternalOutput").ap()

    with tile.TileContext(nc) as tc:
        with tc.tile_pool(name="persist", bufs=1) as persist:
            # ---- small setup tensors ----
            bq_col = persist.tile([128, NDT], F32)
            bk_col = persist.tile([128, NDT], F32)
            nc.sync.dma_start(out=bq_col[:], in_=bass.AP(tensor=bq.tensor, offset=0, ap=[[1, 128], [128, NDT]]))
            nc.sync.dma_start(out=bk_col[:], in_=bass.AP(tensor=bk.tensor, offset=0, ap=[[1, 128], [128, NDT]]))
            bv_b = persist.tile([128, D], F32)
            nc.sync.dma_start(out=bv_b[:], in_=bass.AP(tensor=bv.tensor, offset=0, ap=[[0, 128], [1, D]]))
            xk_b = persist.tile([128, S], F32)
            yk_b = persist.tile([128, S], F32)
            nc.sync.dma_start(out=xk_b[:], in_=bass.AP(tensor=pkx.tensor, offset=0, ap=[[0, 128], [1, S]]))
            nc.sync.dma_start(out=yk_b[:], in_=bass.AP(tensor=pky.tensor, offset=0, ap=[[0, 128], [1, S]]))
            xq_col = persist.tile([128, NQT], F32)
            yq_col = persist.tile([128, NQT], F32)
            nc.sync.dma_start(out=xq_col[:], in_=bass.AP(tensor=pqx.tensor, offset=0, ap=[[1, 128], [128, NQT]]))
            nc.sync.dma_start(out=yq_col[:], in_=bass.AP(tensor=pqy.tensor, offset=0, ap=[[1, 128], [128, NQT]]))
            # emb^T block-diagonal [128, 20]: rows 0-63 head-even, 64-127 head-odd
            embT_blk = persist.tile([128, 2 * NUM_EMB], BF16)
            nc.vector.memset(embT_blk[:], 0.0)
            nc.sync.dma_start(out=embT_blk[0:64, 0:NUM_EMB], in_=embt[:, :])
            nc.sync.dma_start(out=embT_blk[64:128, NUM_EMB:2 * NUM_EMB], in_=embt[:, :])

            ident = persist.tile([128, 128], BF16)
            make_identity(nc, ident[:])

            # ---- persistent big tensors ----
            kT = persist.tile([128, NDT, S], BF16)        # K^T [dim, token]
            qT = persist.tile([128, NDT, SQ], BF16)       # Q^T [dim, token]
            v_sb = persist.tile([128, NKT, H, DK + 1], BF16)  # V [token, head, dk+1]
            nc.vector.memset(v_sb[:, :, :, DK:DK + 1], 1.0)   # ones col -> denominator
            masks_all = persist.tile([128, NQT, n_e, S], BF16)  # step masks per q-tile
            dqe = persist.tile([128, NQT, H, n_e], F32)   # qe band steps

            # ---- step-mask prep: depends only on positions, so it is issued
            # BEFORE the projections and runs on DVE (+2 Act squares) while PE
            # projects and Act handles the projection copies. The pool stays
            # open through the projections (no SBUF-reuse serialization).
            # (Sign-masks on Act were tried: they stall the in-order Act queue
            # ahead of the projection copies and lose ~35us.)
            THRESH2 = [float(((e - 0.5) * MAX_DIST / 9.0) ** 2) for e in range(1, n_e + 1)]
            mprep = tc.tile_pool(name="mprep", bufs=1)
            mp = mprep.__enter__()
            for qt in range(NQT):
                dx = mp.tile([128, S], F32, tag="dx")
                dy = mp.tile([128, S], F32, tag="dy")
                nc.vector.tensor_scalar(out=dx[:], in0=xk_b[:], scalar1=xq_col[:, qt:qt + 1],
                                        scalar2=None, op0=ALU.subtract)
                nc.vector.tensor_scalar(out=dy[:], in0=yk_b[:], scalar1=yq_col[:, qt:qt + 1],
                                        scalar2=None, op0=ALU.subtract)
                dx2 = mp.tile([128, S], F32, tag="dx2")
                dy2 = mp.tile([128, S], F32, tag="dy2")
                nc.scalar.activation(dx2[:], dx[:], ACT.Square)
                nc.scalar.activation(dy2[:], dy[:], ACT.Square)
                d2 = mp.tile([128, S], F32, tag="d2")
                nc.vector.tensor_add(d2[:], dx2[:], dy2[:])
                for e in range(n_e):
                    nc.vector.tensor_scalar(out=masks_all[:, qt, e, :], in0=d2[:],
                                            scalar1=THRESH2[e], scalar2=None,
                                            op0=ALU.is_ge)

            # ---- projections (all bf16; lhsT/rhs host-pre-transposed) ----
            # Order: Q first (qe/dqe depend on it), then K (first heads can
            # start), then V (only needed one head later by the AV stage).
            with tc.tile_pool(name="proj", bufs=1) as pj, \
                 tc.tile_pool(name="pj_ps", bufs=4, space="PSUM") as pj_ps, \
                 tc.tile_pool(name="prep", bufs=1) as prep, \
                 tc.tile_pool(name="qe_ps", bufs=2, space="PSUM") as qe_ps:
                wq_sb = pj.tile([128, NDT, D], BF16)
                xq_sb = pj.tile([128, NDT, SQ], BF16)
                nc.sync.dma_start(out=wq_sb[:], in_=wqt.rearrange("(t p) o -> p t o", p=128))
                nc.sync.dma_start(out=xq_sb[:], in_=xqt.rearrange("(t p) j -> p t j", p=128))
                wk_sb = pj.tile([128, NDT, D], BF16)
                xk_sb = pj.tile([128, NDT, S], BF16)
                nc.sync.dma_start(out=wk_sb[:], in_=wkt.rearrange("(t p) o -> p t o", p=128))
                nc.sync.dma_start(out=xk_sb[:], in_=xkt.rearrange("(t p) j -> p t j", p=128))
                wv_sb = pj.tile([128, NDT, D], BF16)
                xv_sb = pj.tile([128, NDT, S], BF16)
                nc.sync.dma_start(out=wv_sb[:], in_=wvt.rearrange("(t p) o -> p t o", p=128))
                nc.sync.dma_start(out=xv_sb[:], in_=xvt.rearrange("(t p) j -> p t j", p=128))

                for m in range(NDT):
                    ps = pj_ps.tile([128, 512], F32, tag="pj")
                    for t in range(NDT):
                        nc.tensor.matmul(ps[:], wq_sb[:, t, 128 * m:128 * m + 128],
                                         xq_sb[:, t, :],
                                         start=(t == 0), stop=(t == NDT - 1))
                    nc.scalar.activation(qT[:, m, :], ps[:], ACT.Identity,
                                         bias=bq_col[:, m:m + 1])

                # qe -> dqe band steps (only needs qT)
                for qt in range(NQT):
                    qe_psum = qe_ps.tile([128, H * NUM_EMB], F32, tag="qe")
                    for m in range(NDT):
                        nc.tensor.matmul(qe_psum[:, 20 * m:20 * m + 20],
                                         qT[:, m, 128 * qt:128 * qt + 128],
                                         embT_blk[:],
                                         start=True, stop=True)
                    qe_sb = prep.tile([128, H, NUM_EMB], F32, tag="qe_sb")
                    nc.scalar.copy(qe_sb[:], qe_psum[:].rearrange("p (h e) -> p h e", e=NUM_EMB))
                    nc.vector.tensor_tensor(out=dqe[:, qt, :, :],
                                            in0=qe_sb[:, :, 1:1 + n_e],
                                            in1=qe_sb[:, :, 0:n_e], op=ALU.subtract)

                for m in range(NDT):
                    for hf in range(2):
                        ps = pj_ps.tile([128, 512], F32, tag="pj")
                        for t in range(NDT):
                            nc.tensor.matmul(ps[:], wk_sb[:, t, 128 * m:128 * m + 128],
                                             xk_sb[:, t, 512 * hf:512 * hf + 512],
                                             start=(t == 0), stop=(t == NDT - 1))
                        nc.scalar.activation(kT[:, m, 512 * hf:512 * hf + 512], ps[:],
                                             ACT.Identity, bias=bk_col[:, m:m + 1])

                for m in range(NKT):
                    for hf in range(2):
                        ps = pj_ps.tile([128, 384], F32, tag="pj")
                        for t in range(NDT):
                            nc.tensor.matmul(ps[:], xv_sb[:, t, 128 * m:128 * m + 128],
                                             wv_sb[:, t, 384 * hf:384 * hf + 384],
                                             start=(t == 0), stop=(t == NDT - 1))
                        # scatter 6 head slices [128, 6, 64] -> v_sb[:, m, 6hf:6hf+6, 0:64]
                        nc.scalar.copy(
                            v_sb[:, m, 6 * hf:6 * hf + 6, 0:DK],
                            ps[:].rearrange("p (h d) -> p h d", d=DK))

            mprep.__exit__(None, None, None)

            # ---- attention: software-pipelined over (qt, h) ----
            with tc.tile_pool(name="att", bufs=2) as att, \
                 tc.tile_pool(name="osb", bufs=2) as osb, \
                 tc.tile_pool(name="qk_ps", bufs=2, space="PSUM") as qk_ps, \
                 tc.tile_pool(name="pt_ps", bufs=2, space="PSUM") as pt_ps, \
                 tc.tile_pool(name="av_ps", bufs=2, space="PSUM") as av_ps:

                prev = None          # (qt, h, p_sb, o_tile)
                pend = None          # (qt, h, av, o_tile) awaiting normalize
                o_tile = None

                def finish_pe(prev):
                    """transpose P (PE), copy to SBUF (Act), AV matmuls (PE)."""
                    qt_p, h_p, p_sb, o_t = prev
                    ptp = pt_ps.tile([128, NKT, 128], BF16, tag="ptp")
                    for c in range(NKT):
                        nc.tensor.transpose(ptp[:, c, :], p_sb[:, 128 * c:128 * c + 128], ident[:])
                    pT = att.tile([128, NKT, 128], BF16, tag="pT")
                    nc.scalar.copy(pT[:], ptp[:])
                    av = av_ps.tile([128, DK + 1], F32, tag="av")
                    for c in range(NKT):
                        nc.tensor.matmul(av[:], pT[:, c, :], v_sb[:, c, h_p, :],
                                         start=(c == 0), stop=(c == NKT - 1))
                    return (qt_p, h_p, av, o_t)

                def finish_post(pend):
                    """normalize by the ones-column row sum, add bv, store."""
                    qt_p, h_p, av, o_t = pend
                    recip = att.tile([128, 1], F32, tag="recip")
                    nc.vector.reciprocal(recip[:], av[:, DK:DK + 1])
                    nc.vector.scalar_tensor_tensor(
                        out=o_t[:, h_p, :], in0=av[:, 0:DK], scalar=recip[:],
                        in1=bv_b[:, DK * h_p:DK * h_p + DK], op0=ALU.mult, op1=ALU.add)
                    if h_p == H - 1:
                        nc.sync.dma_start(
                            out=out[128 * qt_p:128 * qt_p + 128, :],
                            in_=o_t[:].rearrange("p h d -> p (h d)"))

                for qt in range(NQT):
                    o_tile = osb.tile([128, H, DK], F32, tag="o")
                    for h in range(H):
                        off = (64 * h) % 128
                        # PE leads with prev head's transpose+AV (ready work)
                        if prev is not None:
                            pend = finish_pe(prev)
                        # --- bias band tiles: t_e = mask_e * dqe_e, bf16 4x ---
                        tt = att.tile([128, n_e, S], BF16, tag="tt")
                        for e in range(n_e):
                            nc.vector.tensor_scalar(
                                out=tt[:, e, :], in0=masks_all[:, qt, e, :],
                                scalar1=dqe[:, qt, h, e:e + 1], scalar2=None,
                                op0=ALU.mult)
                        # --- small DVE tree over the first n_dve tiles ---
                        # (GpSimd adds were tried here: they contend with DVE
                        # for SBUF ports and slowed every DVE op ~50%.)
                        r = tt[:, 0, :]
                        for d in range(1, n_dve):
                            racc = att.tile([128, S], BF16, tag=f"racc{d % 2}")
                            nc.vector.tensor_tensor(out=racc[:], in0=r, in1=tt[:, d, :], op=ALU.add)
                            r = racc[:]
                        inj = [tt[:, e, :] for e in range(n_dve, n_e)] + [r]
                        # --- qk + injected bias accumulation in PSUM ---
                        qk = qk_ps.tile([128, S], F32, tag="qk")
                        for hf in range(2):
                            sl = slice(512 * hf, 512 * hf + 512)
                            nc.tensor.matmul(qk[:, sl],
                                             qT[off:off + 64, h // 2, 128 * qt:128 * qt + 128],
                                             kT[off:off + 64, h // 2, sl],
                                             start=True, stop=False)
                            for ii, tsl in enumerate(inj):
                                nc.tensor.matmul(qk[:, sl], ident[:], tsl[:, sl],
                                                 start=False, stop=(ii == len(inj) - 1))
                        # --- P = exp(logits/8) ---
                        p_sb = att.tile([128, S], BF16, tag="p")
                        nc.scalar.activation(p_sb[:], qk[:], ACT.Exp, scale=0.125)
                        if pend is not None:
                            finish_post(pend)
                            pend = None
                        prev = (qt, h, p_sb, o_tile)
                # drain
                finish_post(finish_pe(prev))
    nc.compile()
    return nc


_NC_CACHE = {}


def _get_nc(n_e=None):
    if n_e is None:
        n_e = _NC_CACHE.get("last", 8)
    if n_e not in _NC_CACHE:
        _NC_CACHE[n_e] = build_nc(n_e=n_e)
    _NC_CACHE["last"] = n_e
    return _NC_CACHE[n_e]


def _make_in_maps(inputs):
    query = np.asarray(inputs["query"], dtype=np.float32)
    key = np.asarray(inputs["key"], dtype=np.float32)
    value = np.asarray(inputs["value"], dtype=np.float32)
    tp = np.asarray(inputs["tile_positions"], dtype=np.float32)
    Wq = np.asarray(inputs["Wq"], dtype=np.float32)
    Wk = np.asarray(inputs["Wk"], dtype=np.float32)
    Wv = np.asarray(inputs["Wv"], dtype=np.float32)
    bq = np.asarray(inputs["bq"], dtype=np.float32)
    bk = np.asarray(inputs["bk"], dtype=np.float32)
    bv = np.asarray(inputs["bv"], dtype=np.float32)
    emb = np.asarray(inputs["emb_k"], dtype=np.float32)

    wqt = np.ascontiguousarray(Wq.T.astype(BF))
    wkt = np.ascontiguousarray(Wk.T.astype(BF))
    wvt = np.ascontiguousarray(Wv.T.astype(BF))
    embt = np.ascontiguousarray(emb.T.astype(BF))

    in_maps = []
    for c in range(NCORES):
        b, qh = c // 2, c % 2
        sl = slice(qh * SQ, (qh + 1) * SQ)
        in_maps.append({
            "xqt": np.ascontiguousarray(query[b, sl].T.astype(BF)),
            "xkt": np.ascontiguousarray(key[b].T.astype(BF)),
            "xvt": np.ascontiguousarray(value[b].T.astype(BF)),
            "wqt": wqt, "wkt": wkt, "wvt": wvt, "embt": embt,
            "bq": bq, "bk": bk, "bv": bv,
            "pkx": np.ascontiguousarray(tp[b, :, 0]),
            "pky": np.ascontiguousarray(tp[b, :, 1]),
            "pqx": np.ascontiguousarray(tp[b, sl, 0]),
            "pqy": np.ascontiguousarray(tp[b, sl, 1]),
        })
    return in_maps


def _active_bands(tp):
    """Highest band index that actually occurs for these positions."""
    mx = 0.0
    for b in range(tp.shape[0]):
        p = tp[b]
        d2 = ((p[:, None, :] - p[None, :, :]) ** 2).sum(-1)
        mx = max(mx, float(d2.max()))
    max_idx = int(np.floor(9.0 * np.sqrt(mx) / MAX_DIST + 0.5))
    return max(1, min(max_idx, NUM_EMB - 1))


def kernel(query, key, value, tile_positions, Wq, bq, Wk, bk, Wv, bv, emb_k):
    inputs = {"query": query, "key": key, "value": value,
              "tile_positions": tile_positions,
              "Wq": Wq, "bq": bq, "Wk": Wk, "bk": bk, "Wv": Wv, "bv": bv,
              "emb_k": emb_k}
    tp = np.asarray(tile_positions, dtype=np.float32)
    n_e = _active_bands(tp)
    nc = _get_nc(n_e)
    in_maps = _make_in_maps(inputs)
    res = run_bass_kernel_spmd(nc, in_maps, core_ids=list(range(NCORES)))
    out = np.empty((B, S, D), np.float32)
    for c in range(NCORES):
        b, qh = c // 2, c % 2
        out[b, qh * SQ:(qh + 1) * SQ] = res.results[c]["out"]
    return out


# revision 29
# speedup vs baseline: 1.1620x; 1.0624x over previous
"""Distance-aware multihead attention on 8 Trainium2 NeuronCores.

Problem: B=4, S=1024, D=768, H=12, DK=64, NUM_EMB=10.
  q/k/v = linear projections of query/key/value
  idx[b,i,j] = clip(round(9 * |pos_i - pos_j| / MAXD), 0, 9)
  logits = (q.k^T + qe[b,h,i,idx[b,i,j]]) / 8   where qe = q @ emb_k^T
  out = softmax(logits) @ v

Design:
  - All matmul inputs are bf16, host-pre-transposed so the device does ZERO
    input transposes (the baseline lost ~3.1ms of 3.5ms to per-element DMA
    descriptors from 4-byte dma transposes).
  - Step masks (d2 >= T_e^2, fp32-exact compare, bf16 0/1 output) are built
    once per q-tile on DVE, issued BEFORE the projections so they overlap the
    PE-bound startup. The bias decomposes as qe[idx] - qe[0] =
    sum_{e=1..E} dqe_e * mask_e; per (head, q-tile) each band term is ONE
    tensor_scalar mult with a per-partition dqe scalar, running in the DVE 4x
    perf mode on bf16 (~395ns/tile; scalar_tensor_tensor has NO fast mode,
    which is why the baseline's STT chain was 6x slower).
  - Band tiles are merged into the QK PSUM partly via a small DVE add tree
    (n_dve) and partly via identity-matmul accumulation on TensorE
    (psum += I @ t_e, ~170ns per [128,512] half).
  - E is data-adaptive: bands that cannot occur for the given positions are
    dropped at build time (seed-0 data has max idx 8, so E=8).
  - Softmax denominator comes free from a ones-column appended to each V head
    slice in the AV matmul; P^T comes from PE transposes with the PSUM->SBUF
    copy on the scalar engine.
  - The (qt, h) loop is software-pipelined: PE leads each iteration with the
    previous head's transpose+AV so it never stalls on the current head's
    DVE band tiles.
  - Engines measured ~: DVE 85%, PE 90%, Act 43% in steady state.
  - Rejected variants (measured slower): GpSimd tree adds (SBUF port
    contention slows all DVE ops ~50%); Sign-masks / band-mults on the
    scalar engine (stalls the in-order Act queue ahead of projection copies
    and the exp chain).

Sharding: core c handles batch c//2, query-half c%2 (512 queries, all heads).
"""
import numpy as np
import ml_dtypes

import concourse.bass as bass
import concourse.tile as tile
from concourse import bacc, mybir
from concourse.bass_utils import run_bass_kernel_spmd
from concourse.masks import make_identity

F32 = mybir.dt.float32
BF16 = mybir.dt.bfloat16
ACT = mybir.ActivationFunctionType
ALU = mybir.AluOpType

B, S, D = 4, 1024, 768
H, DK = 12, 64
NUM_EMB = 10
MAX_DIST = 100000.0 * 2 ** 0.5
SQ = S // 2          # queries per core
NQT = SQ // 128      # q-tiles per core (4)
NKT = S // 128       # k token chunks (8)
NDT = D // 128       # dim tiles (6)
NCORES = 8
SCL9 = 9.0 / MAX_DIST

BF = ml_dtypes.bfloat16


def build_nc(n_e=8, n_dve=3):
    """n_e: number of active bias bands (e = 1..n_e).
    n_dve: how many t_e tiles are merged by a DVE add tree; the rest (and the
    tree root) are accumulated into the QK psum by identity matmuls on PE."""
    nc = bacc.Bacc("TRN2", target_bir_lowering=False, debug=False)

    xqt = nc.dram_tensor("xqt", [D, SQ], BF16, kind="ExternalInput").ap()
    xkt = nc.dram_tensor("xkt", [D, S], BF16, kind="ExternalInput").ap()
    xvt = nc.dram_tensor("xvt", [D, S], BF16, kind="ExternalInput").ap()
    wqt = nc.dram_tensor("wqt", [D, D], BF16, kind="ExternalInput").ap()
    wkt = nc.dram_tensor("wkt", [D, D], BF16, kind="ExternalInput").ap()
    wvt = nc.dram_tensor("wvt", [D, D], BF16, kind="ExternalInput").ap()
    embt = nc.dram_tensor("embt", [DK, NUM_EMB], BF16, kind="ExternalInput").ap()
    bq = nc.dram_tensor("bq", [D], F32, kind="ExternalInput").ap()
    bk = nc.dram_tensor("bk", [D], F32, kind="ExternalInput").ap()
    bv = nc.dram_tensor("bv", [D], F32, kind="ExternalInput").ap()
    pkx = nc.dram_tensor("pkx", [S], F32, kind="ExternalInput").ap()
    pky = nc.dram_tensor("pky", [S], F32, kind="ExternalInput").ap()
    pqx = nc.dram_tensor("pqx", [SQ], F32, kind="ExternalInput").ap()
    pqy = nc.dram_tensor("pqy", [SQ], F32, kind="ExternalInput").ap()
    out = nc.dram_tensor("out", [SQ, D], F32, kind="ExternalOutput").ap()

    with tile.TileContext(nc) as tc:
        with tc.tile_pool(name="persist", bufs=1) as persist:
            # ---- small setup tensors ----
            bq_col = persist.tile([128, NDT], F32)
            bk_col = persist.tile([128, NDT], F32)
            nc.sync.dma_start(out=bq_col[:], in_=bass.AP(tensor=bq.tensor, offset=0, ap=[[1, 128], [128, NDT]]))
            nc.sync.dma_start(out=bk_col[:], in_=bass.AP(tensor=bk.tensor, offset=0, ap=[[1, 128], [128, NDT]]))
            bv_b = persist.tile([128, D], F32)
            nc.sync.dma_start(out=bv_b[:], in_=bass.AP(tensor=bv.tensor, offset=0, ap=[[0, 128], [1, D]]))
            xk_b = persist.tile([128, S], F32)
            yk_b = persist.tile([128, S], F32)
            nc.sync.dma_start(out=xk_b[:], in_=bass.AP(tensor=pkx.tensor, offset=0, ap=[[0, 128], [1, S]]))
            nc.sync.dma_start(out=yk_b[:], in_=bass.AP(tensor=pky.tensor, offset=0, ap=[[0, 128], [1, S]]))
            xq_col = persist.tile([128, NQT], F32)
            yq_col = persist.tile([128, NQT], F32)
            nc.sync.dma_start(out=xq_col[:], in_=bass.AP(tensor=pqx.tensor, offset=0, ap=[[1, 128], [128, NQT]]))
            nc.sync.dma_start(out=yq_col[:], in_=bass.AP(tensor=pqy.tensor, offset=0, ap=[[1, 128], [128, NQT]]))
            # emb^T block-diagonal [128, 20]: rows 0-63 head-even, 64-127 head-odd
            embT_blk = persist.tile([128, 2 * NUM_EMB], BF16)
            nc.vector.memset(embT_blk[:], 0.0)
            nc.sync.dma_start(out=embT_blk[0:64, 0:NUM_EMB], in_=embt[:, :])
            nc.sync.dma_start(out=embT_blk[64:128, NUM_EMB:2 * NUM_EMB], in_=embt[:, :])

            ident = persist.tile([128, 128], BF16)
            make_identity(nc, ident[:])

            # ---- persistent big tensors ----
            kT = persist.tile([128, NDT, S], BF16)        # K^T [dim, token]
            qT = persist.tile([128, NDT, SQ], BF16)       # Q^T [dim, token]
            v_sb = persist.tile([128, NKT, H, DK + 1], BF16)  # V [token, head, dk+1]
            nc.vector.memset(v_sb[:, :, :, DK:DK + 1], 1.0)   # ones col -> denominator
            masks_all = persist.tile([128, NQT, n_e, S], BF16)  # step masks per q-tile
            dqe = persist.tile([128, NQT, H, n_e], F32)   # qe band steps

            # ---- step-mask prep: depends only on positions, so it is issued
            # BEFORE the projections and runs on DVE (+2 Act squares) while PE
            # projects and Act handles the projection copies. The pool stays
            # open through the projections (no SBUF-reuse serialization).
            # (Sign-masks on Act were tried: they stall the in-order Act queue
            # ahead of the projection copies and lose ~35us.)
            THRESH2 = [float(((e - 0.5) * MAX_DIST / 9.0) ** 2) for e in range(1, n_e + 1)]
            mprep = tc.tile_pool(name="mprep", bufs=1)
            mp = mprep.__enter__()
            for qt in range(NQT):
                dx = mp.tile([128, S], F32, tag="dx")
                dy = mp.tile([128, S], F32, tag="dy")
                nc.vector.tensor_scalar(out=dx[:], in0=xk_b[:], scalar1=xq_col[:, qt:qt + 1],
                                        scalar2=None, op0=ALU.subtract)
                nc.vector.tensor_scalar(out=dy[:], in0=yk_b[:], scalar1=yq_col[:, qt:qt + 1],
                                        scalar2=None, op0=ALU.subtract)
                dx2 = mp.tile([128, S], F32, tag="dx2")
                dy2 = mp.tile([128, S], F32, tag="dy2")
                nc.scalar.activation(dx2[:], dx[:], ACT.Square)
                nc.scalar.activation(dy2[:], dy[:], ACT.Square)
                d2 = mp.tile([128, S], F32, tag="d2")
                nc.vector.tensor_add(d2[:], dx2[:], dy2[:])
                for e in range(n_e):
                    nc.vector.tensor_scalar(out=masks_all[:, qt, e, :], in0=d2[:],
                                            scalar1=THRESH2[e], scalar2=None,
                                            op0=ALU.is_ge)

            # ---- projections (all bf16; lhsT/rhs host-pre-transposed) ----
            # Order: Q first (qe/dqe depend on it), then K (first heads can
            # start), then V (only needed one head later by the AV stage).
            with tc.tile_pool(name="proj", bufs=1) as pj, \
                 tc.tile_pool(name="pj_ps", bufs=4, space="PSUM") as pj_ps, \
                 tc.tile_pool(name="prep", bufs=1) as prep, \
                 tc.tile_pool(name="qe_ps", bufs=2, space="PSUM") as qe_ps:
                wq_sb = pj.tile([128, NDT, D], BF16)
                xq_sb = pj.tile([128, NDT, SQ], BF16)
                nc.sync.dma_start(out=wq_sb[:], in_=wqt.rearrange("(t p) o -> p t o", p=128))
                nc.sync.dma_start(out=xq_sb[:], in_=xqt.rearrange("(t p) j -> p t j", p=128))
                wk_sb = pj.tile([128, NDT, D], BF16)
                xk_sb = pj.tile([128, NDT, S], BF16)
                nc.sync.dma_start(out=wk_sb[:], in_=wkt.rearrange("(t p) o -> p t o", p=128))
                nc.sync.dma_start(out=xk_sb[:], in_=xkt.rearrange("(t p) j -> p t j", p=128))
                wv_sb = pj.tile([128, NDT, D], BF16)
                xv_sb = pj.tile([128, NDT, S], BF16)
                nc.sync.dma_start(out=wv_sb[:], in_=wvt.rearrange("(t p) o -> p t o", p=128))
                nc.sync.dma_start(out=xv_sb[:], in_=xvt.rearrange("(t p) j -> p t j", p=128))

                for m in range(NDT):
                    ps = pj_ps.tile([128, 512], F32, tag="pj")
                    for t in range(NDT):
                        nc.tensor.matmul(ps[:], wq_sb[:, t, 128 * m:128 * m + 128],
                                         xq_sb[:, t, :],
                                         start=(t == 0), stop=(t == NDT - 1))
                    nc.scalar.activation(qT[:, m, :], ps[:], ACT.Identity,
                                         bias=bq_col[:, m:m + 1])

                # qe -> dqe band steps (only needs qT)
                for qt in range(NQT):
                    qe_psum = qe_ps.tile([128, H * NUM_EMB], F32, tag="qe")
                    for m in range(NDT):
                        nc.tensor.matmul(qe_psum[:, 20 * m:20 * m + 20],
                                         qT[:, m, 128 * qt:128 * qt + 128],
                                         embT_blk[:],
                                         start=True, stop=True)
                    qe_sb = prep.tile([128, H, NUM_EMB], F32, tag="qe_sb")
                    nc.scalar.copy(qe_sb[:], qe_psum[:].rearrange("p (h e) -> p h e", e=NUM_EMB))
                    nc.vector.tensor_tensor(out=dqe[:, qt, :, :],
                                            in0=qe_sb[:, :, 1:1 + n_e],
                                            in1=qe_sb[:, :, 0:n_e], op=ALU.subtract)

                for m in range(NDT):
                    for hf in range(2):
                        ps = pj_ps.tile([128, 512], F32, tag="pj")
                        for t in range(NDT):
                            nc.tensor.matmul(ps[:], wk_sb[:, t, 128 * m:128 * m + 128],
                                             xk_sb[:, t, 512 * hf:512 * hf + 512],
                                             start=(t == 0), stop=(t == NDT - 1))
                        nc.scalar.activation(kT[:, m, 512 * hf:512 * hf + 512], ps[:],
                                             ACT.Identity, bias=bk_col[:, m:m + 1])

                for m in range(NKT):
                    for hf in range(2):
                        ps = pj_ps.tile([128, 384], F32, tag="pj")
                        for t in range(NDT):
                            nc.tensor.matmul(ps[:], xv_sb[:, t, 128 * m:128 * m + 128],
                                             wv_sb[:, t, 384 * hf:384 * hf + 384],
                                             start=(t == 0), stop=(t == NDT - 1))
                        # scatter 6 head slices [128, 6, 64] -> v_sb[:, m, 6hf:6hf+6, 0:64]
                        nc.scalar.copy(
                            v_sb[:, m, 6 * hf:6 * hf + 6, 0:DK],
                            ps[:].rearrange("p (h d) -> p h d", d=DK))

            mprep.__exit__(None, None, None)

            # ---- attention: software-pipelined over (qt, h) ----
            with tc.tile_pool(name="att", bufs=2) as att, \
                 tc.tile_pool(name="osb", bufs=2) as osb, \
                 tc.tile_pool(name="qk_ps", bufs=2, space="PSUM") as qk_ps, \
                 tc.tile_pool(name="pt_ps", bufs=2, space="PSUM") as pt_ps, \
                 tc.tile_pool(name="av_ps", bufs=2, space="PSUM") as av_ps:

                prev = None          # (qt, h, p_sb, o_tile)
                pend = None          # (qt, h, av, o_tile) awaiting normalize
                o_tile = None

                def finish_pe(prev):
                    """transpose P (PE), copy to SBUF (Act), AV matmuls (PE)."""
                    qt_p, h_p, p_sb, o_t = prev
                    ptp = pt_ps.tile([128, NKT, 128], BF16, tag="ptp")
                    for c in range(NKT):
                        nc.tensor.transpose(ptp[:, c, :], p_sb[:, 128 * c:128 * c + 128], ident[:])
                    pT = att.tile([128, NKT, 128], BF16, tag="pT")
                    nc.scalar.copy(pT[:], ptp[:])
                    av = av_ps.tile([128, DK + 1], F32, tag="av")
                    for c in range(NKT):
                        nc.tensor.matmul(av[:], pT[:, c, :], v_sb[:, c, h_p, :],
                                         start=(c == 0), stop=(c == NKT - 1))
                    return (qt_p, h_p, av, o_t)

                def finish_post(pend):
                    """normalize by the ones-column row sum, add bv, store."""
                    qt_p, h_p, av, o_t = pend
                    recip = att.tile([128, 1], F32, tag="recip")
                    nc.vector.reciprocal(recip[:], av[:, DK:DK + 1])
                    nc.vector.scalar_tensor_tensor(
                        out=o_t[:, h_p, :], in0=av[:, 0:DK], scalar=recip[:],
                        in1=bv_b[:, DK * h_p:DK * h_p + DK], op0=ALU.mult, op1=ALU.add)
                    if h_p == H - 1:
                        nc.sync.dma_start(
                            out=out[128 * qt_p:128 * qt_p + 128, :],
                            in_=o_t[:].rearrange("p h d -> p (h d)"))

                # n_act band tiles per head are produced on the scalar engine,
                # PREFETCHED one head ahead so the in-order Act queue never
                # stalls the injects (issuing them same-head cost ~13us in v5).
                n_act = 1

                def make_tt(qt_n, h_n):
                    tt_n = att.tile([128, n_e, S], BF16, tag="tt", bufs=3)
                    for e in range(n_e - n_act, n_e):
                        nc.scalar.activation(tt_n[:, e, :], masks_all[:, qt_n, e, :],
                                             ACT.Identity,
                                             scale=dqe[:, qt_n, h_n, e:e + 1])
                    return tt_n

                pairs = [(qt, h) for qt in range(NQT) for h in range(H)]
                tt = None
                for idx, (qt, h) in enumerate(pairs):
                    if h == 0:
                        o_tile = osb.tile([128, H, DK], F32, tag="o")
                    off = (64 * h) % 128
                    if tt is None:
                        tt = make_tt(qt, h)
                    # prefetch next head's Act band tiles
                    tt_nxt = make_tt(*pairs[idx + 1]) if idx + 1 < len(pairs) else None
                    # PE leads with prev head's transpose+AV (ready work)
                    if prev is not None:
                        pend = finish_pe(prev)
                    # --- bias band tiles: t_e = mask_e * dqe_e, bf16 4x ---
                    for e in range(n_e - n_act):
                        nc.vector.tensor_scalar(
                            out=tt[:, e, :], in0=masks_all[:, qt, e, :],
                            scalar1=dqe[:, qt, h, e:e + 1], scalar2=None,
                            op0=ALU.mult)
                    # --- small DVE tree over the first n_dve tiles ---
                    # (GpSimd adds were tried here: they contend with DVE
                    # for SBUF ports and slowed every DVE op ~50%.)
                    r = tt[:, 0, :]
                    for d in range(1, n_dve):
                        racc = att.tile([128, S], BF16, tag=f"racc{d % 2}")
                        nc.vector.tensor_tensor(out=racc[:], in0=r, in1=tt[:, d, :], op=ALU.add)
                        r = racc[:]
                    inj = [tt[:, e, :] for e in range(n_dve, n_e)] + [r]
                    # --- qk + injected bias accumulation in PSUM ---
                    qk = qk_ps.tile([128, S], F32, tag="qk")
                    for hf in range(2):
                        sl = slice(512 * hf, 512 * hf + 512)
                        nc.tensor.matmul(qk[:, sl],
                                         qT[off:off + 64, h // 2, 128 * qt:128 * qt + 128],
                                         kT[off:off + 64, h // 2, sl],
                                         start=True, stop=False)
                        for ii, tsl in enumerate(inj):
                            nc.tensor.matmul(qk[:, sl], ident[:], tsl[:, sl],
                                             start=False, stop=(ii == len(inj) - 1))
                    # --- P = exp(logits/8) ---
                    p_sb = att.tile([128, S], BF16, tag="p")
                    nc.scalar.activation(p_sb[:], qk[:], ACT.Exp, scale=0.125)
                    if pend is not None:
                        finish_post(pend)
                        pend = None
                    prev = (qt, h, p_sb, o_tile)
                    tt = tt_nxt
                # drain
                finish_post(finish_pe(prev))
    nc.compile()
    return nc


_NC_CACHE = {}


def _get_nc(n_e=None):
    if n_e is None:
        n_e = _NC_CACHE.get("last", 8)
    if n_e not in _NC_CACHE:
        _NC_CACHE[n_e] = build_nc(n_e=n_e)
    _NC_CACHE["last"] = n_e
    return _NC_CACHE[n_e]


def _make_in_maps(inputs):
    query = np.asarray(inputs["query"], dtype=np.float32)
    key = np.asarray(inputs["key"], dtype=np.float32)
    value = np.asarray(inputs["value"], dtype=np.float32)
    tp = np.asarray(inputs["tile_positions"], dtype=np.float32)
    Wq = np.asarray(inputs["Wq"], dtype=np.float32)
    Wk = np.asarray(inputs["Wk"], dtype=np.float32)
    Wv = np.asarray(inputs["Wv"], dtype=np.float32)
    bq = np.asarray(inputs["bq"], dtype=np.float32)
    bk = np.asarray(inputs["bk"], dtype=np.float32)
    bv = np.asarray(inputs["bv"], dtype=np.float32)
    emb = np.asarray(inputs["emb_k"], dtype=np.float32)

    wqt = np.ascontiguousarray(Wq.T.astype(BF))
    wkt = np.ascontiguousarray(Wk.T.astype(BF))
    wvt = np.ascontiguousarray(Wv.T.astype(BF))
    embt = np.ascontiguousarray(emb.T.astype(BF))

    in_maps = []
    for c in range(NCORES):
        b, qh = c // 2, c % 2
        sl = slice(qh * SQ, (qh + 1) * SQ)
        in_maps.append({
            "xqt": np.ascontiguousarray(query[b, sl].T.astype(BF)),
            "xkt": np.ascontiguousarray(key[b].T.astype(BF)),
            "xvt": np.ascontiguousarray(value[b].T.astype(BF)),
            "wqt": wqt, "wkt": wkt, "wvt": wvt, "embt": embt,
            "bq": bq, "bk": bk, "bv": bv,
            "pkx": np.ascontiguousarray(tp[b, :, 0]),
            "pky": np.ascontiguousarray(tp[b, :, 1]),
            "pqx": np.ascontiguousarray(tp[b, sl, 0]),
            "pqy": np.ascontiguousarray(tp[b, sl, 1]),
        })
    return in_maps


def _active_bands(tp):
    """Highest band index that actually occurs for these positions."""
    mx = 0.0
    for b in range(tp.shape[0]):
        p = tp[b]
        d2 = ((p[:, None, :] - p[None, :, :]) ** 2).sum(-1)
        mx = max(mx, float(d2.max()))
    max_idx = int(np.floor(9.0 * np.sqrt(mx) / MAX_DIST + 0.5))
    return max(1, min(max_idx, NUM_EMB - 1))


def kernel(query, key, value, tile_positions, Wq, bq, Wk, bk, Wv, bv, emb_k):
    inputs = {"query": query, "key": key, "value": value,
              "tile_positions": tile_positions,
              "Wq": Wq, "bq": bq, "Wk": Wk, "bk": bk, "Wv": Wv, "bv": bv,
              "emb_k": emb_k}
    tp = np.asarray(tile_positions, dtype=np.float32)
    n_e = _active_bands(tp)
    nc = _get_nc(n_e)
    in_maps = _make_in_maps(inputs)
    res = run_bass_kernel_spmd(nc, in_maps, core_ids=list(range(NCORES)))
    out = np.empty((B, S, D), np.float32)
    for c in range(NCORES):
        b, qh = c // 2, c % 2
        out[b, qh * SQ:(qh + 1) * SQ] = res.results[c]["out"]
    return out


# revision 30
# speedup vs baseline: 1.2001x; 1.0328x over previous
"""Distance-aware multihead attention on 8 Trainium2 NeuronCores.

Problem: B=4, S=1024, D=768, H=12, DK=64, NUM_EMB=10.
  q/k/v = linear projections of query/key/value
  idx[b,i,j] = clip(round(9 * |pos_i - pos_j| / MAXD), 0, 9)
  logits = (q.k^T + qe[b,h,i,idx[b,i,j]]) / 8   where qe = q @ emb_k^T
  out = softmax(logits) @ v

Design:
  - All matmul inputs are bf16, host-pre-transposed so the device does ZERO
    input transposes (the baseline lost ~3.1ms of 3.5ms to per-element DMA
    descriptors from 4-byte dma transposes).
  - Step masks (d2 >= T_e^2, fp32-exact compare, bf16 0/1 output) are built
    once per q-tile on DVE, issued BEFORE the projections so they overlap the
    PE-bound startup. The bias decomposes as qe[idx] - qe[0] =
    sum_{e=1..E} dqe_e * mask_e; per (head, q-tile) each band term is ONE
    tensor_scalar mult with a per-partition dqe scalar, running in the DVE 4x
    perf mode on bf16 (~395ns/tile; scalar_tensor_tensor has NO fast mode,
    which is why the baseline's STT chain was 6x slower).
  - Band tiles are merged into the QK PSUM partly via a small DVE add tree
    (n_dve) and partly via identity-matmul accumulation on TensorE
    (psum += I @ t_e, ~170ns per [128,512] half).
  - E is data-adaptive: bands that cannot occur for the given positions are
    dropped at build time (seed-0 data has max idx 8, so E=8).
  - Softmax denominator comes free from a ones-column appended to each V head
    slice in the AV matmul; P^T comes from PE transposes with the PSUM->SBUF
    copy on the scalar engine.
  - The (qt, h) loop is software-pipelined: PE leads each iteration with the
    previous head's transpose+AV so it never stalls on the current head's
    DVE band tiles.
  - Engines measured ~: DVE 85%, PE 90%, Act 43% in steady state.
  - Rejected variants (measured slower): GpSimd tree adds (SBUF port
    contention slows all DVE ops ~50%); Sign-masks / band-mults on the
    scalar engine (stalls the in-order Act queue ahead of projection copies
    and the exp chain).

Sharding: core c handles batch c//2, query-half c%2 (512 queries, all heads).
"""
import numpy as np
import ml_dtypes

import concourse.bass as bass
import concourse.tile as tile
from concourse import bacc, mybir
from concourse.bass_utils import run_bass_kernel_spmd
from concourse.masks import make_identity

F32 = mybir.dt.float32
BF16 = mybir.dt.bfloat16
ACT = mybir.ActivationFunctionType
ALU = mybir.AluOpType

B, S, D = 4, 1024, 768
H, DK = 12, 64
NUM_EMB = 10
MAX_DIST = 100000.0 * 2 ** 0.5
SQ = S // 2          # queries per core
NQT = SQ // 128      # q-tiles per core (4)
NKT = S // 128       # k token chunks (8)
NDT = D // 128       # dim tiles (6)
NCORES = 8
SCL9 = 9.0 / MAX_DIST

BF = ml_dtypes.bfloat16


def build_nc(n_e=8, n_dve=3):
    """n_e: number of active bias bands (e = 1..n_e).
    n_dve: how many t_e tiles are merged by a DVE add tree; the rest (and the
    tree root) are accumulated into the QK psum by identity matmuls on PE."""
    nc = bacc.Bacc("TRN2", target_bir_lowering=False, debug=False)

    xqt = nc.dram_tensor("xqt", [D, SQ], BF16, kind="ExternalInput").ap()
    xkt = nc.dram_tensor("xkt", [D, S], BF16, kind="ExternalInput").ap()
    xvt = nc.dram_tensor("xvt", [D, S], BF16, kind="ExternalInput").ap()
    wqt = nc.dram_tensor("wqt", [D, D], BF16, kind="ExternalInput").ap()
    wkt = nc.dram_tensor("wkt", [D, D], BF16, kind="ExternalInput").ap()
    wvt = nc.dram_tensor("wvt", [D, D], BF16, kind="ExternalInput").ap()
    embt = nc.dram_tensor("embt", [DK, NUM_EMB], BF16, kind="ExternalInput").ap()
    bq = nc.dram_tensor("bq", [D], F32, kind="ExternalInput").ap()
    bk = nc.dram_tensor("bk", [D], F32, kind="ExternalInput").ap()
    bv = nc.dram_tensor("bv", [D], F32, kind="ExternalInput").ap()
    pkx = nc.dram_tensor("pkx", [S], F32, kind="ExternalInput").ap()
    pky = nc.dram_tensor("pky", [S], F32, kind="ExternalInput").ap()
    pqx = nc.dram_tensor("pqx", [SQ], F32, kind="ExternalInput").ap()
    pqy = nc.dram_tensor("pqy", [SQ], F32, kind="ExternalInput").ap()
    out = nc.dram_tensor("out", [SQ, D], F32, kind="ExternalOutput").ap()

    with tile.TileContext(nc) as tc:
        with tc.tile_pool(name="persist", bufs=1) as persist:
            # ---- small setup tensors ----
            bq_col = persist.tile([128, NDT], F32)
            bk_col = persist.tile([128, NDT], F32)
            nc.sync.dma_start(out=bq_col[:], in_=bass.AP(tensor=bq.tensor, offset=0, ap=[[1, 128], [128, NDT]]))
            nc.sync.dma_start(out=bk_col[:], in_=bass.AP(tensor=bk.tensor, offset=0, ap=[[1, 128], [128, NDT]]))
            bv_b = persist.tile([128, D], F32)
            nc.sync.dma_start(out=bv_b[:], in_=bass.AP(tensor=bv.tensor, offset=0, ap=[[0, 128], [1, D]]))
            xk_b = persist.tile([128, S], F32)
            yk_b = persist.tile([128, S], F32)
            nc.sync.dma_start(out=xk_b[:], in_=bass.AP(tensor=pkx.tensor, offset=0, ap=[[0, 128], [1, S]]))
            nc.sync.dma_start(out=yk_b[:], in_=bass.AP(tensor=pky.tensor, offset=0, ap=[[0, 128], [1, S]]))
            xq_col = persist.tile([128, NQT], F32)
            yq_col = persist.tile([128, NQT], F32)
            nc.sync.dma_start(out=xq_col[:], in_=bass.AP(tensor=pqx.tensor, offset=0, ap=[[1, 128], [128, NQT]]))
            nc.sync.dma_start(out=yq_col[:], in_=bass.AP(tensor=pqy.tensor, offset=0, ap=[[1, 128], [128, NQT]]))
            # emb^T block-diagonal [128, 20]: rows 0-63 head-even, 64-127 head-odd
            embT_blk = persist.tile([128, 2 * NUM_EMB], BF16)
            nc.vector.memset(embT_blk[:], 0.0)
            nc.sync.dma_start(out=embT_blk[0:64, 0:NUM_EMB], in_=embt[:, :])
            nc.sync.dma_start(out=embT_blk[64:128, NUM_EMB:2 * NUM_EMB], in_=embt[:, :])

            ident = persist.tile([128, 128], BF16)
            make_identity(nc, ident[:])

            # ---- persistent big tensors ----
            kT = persist.tile([128, NDT, S], BF16)        # K^T [dim, token]
            qT = persist.tile([128, NDT, SQ], BF16)       # Q^T [dim, token]
            v_sb = persist.tile([128, NKT, H, DK + 1], BF16)  # V [token, head, dk+1]
            nc.vector.memset(v_sb[:, :, :, DK:DK + 1], 1.0)   # ones col -> denominator
            masks_all = persist.tile([128, NQT, n_e, S], BF16)  # step masks per q-tile
            dqe = persist.tile([128, NQT, H, n_e], F32)   # qe band steps

            # ---- step-mask prep: depends only on positions, so it is issued
            # BEFORE the projections and runs on DVE (+2 Act squares) while PE
            # projects and Act handles the projection copies. The pool stays
            # open through the projections (no SBUF-reuse serialization).
            # (Sign-masks on Act were tried: they stall the in-order Act queue
            # ahead of the projection copies and lose ~35us.)
            THRESH2 = [float(((e - 0.5) * MAX_DIST / 9.0) ** 2) for e in range(1, n_e + 1)]
            mprep = tc.tile_pool(name="mprep", bufs=1)
            mp = mprep.__enter__()
            for qt in range(NQT):
                dx = mp.tile([128, S], F32, tag="dx")
                dy = mp.tile([128, S], F32, tag="dy")
                nc.vector.tensor_scalar(out=dx[:], in0=xk_b[:], scalar1=xq_col[:, qt:qt + 1],
                                        scalar2=None, op0=ALU.subtract)
                nc.vector.tensor_scalar(out=dy[:], in0=yk_b[:], scalar1=yq_col[:, qt:qt + 1],
                                        scalar2=None, op0=ALU.subtract)
                dx2 = mp.tile([128, S], F32, tag="dx2")
                dy2 = mp.tile([128, S], F32, tag="dy2")
                nc.scalar.activation(dx2[:], dx[:], ACT.Square)
                nc.scalar.activation(dy2[:], dy[:], ACT.Square)
                d2 = mp.tile([128, S], F32, tag="d2")
                nc.vector.tensor_add(d2[:], dx2[:], dy2[:])
                for e in range(n_e):
                    nc.vector.tensor_scalar(out=masks_all[:, qt, e, :], in0=d2[:],
                                            scalar1=THRESH2[e], scalar2=None,
                                            op0=ALU.is_ge)

            # ---- projections (all bf16; lhsT/rhs host-pre-transposed) ----
            # Order: Q first (qe/dqe depend on it), then K (first heads can
            # start), then V (only needed one head later by the AV stage).
            with tc.tile_pool(name="proj", bufs=1) as pj, \
                 tc.tile_pool(name="pj_ps", bufs=4, space="PSUM") as pj_ps, \
                 tc.tile_pool(name="prep", bufs=1) as prep, \
                 tc.tile_pool(name="qe_ps", bufs=2, space="PSUM") as qe_ps:
                wq_sb = pj.tile([128, NDT, D], BF16)
                xq_sb = pj.tile([128, NDT, SQ], BF16)
                nc.sync.dma_start(out=wq_sb[:], in_=wqt.rearrange("(t p) o -> p t o", p=128))
                nc.sync.dma_start(out=xq_sb[:], in_=xqt.rearrange("(t p) j -> p t j", p=128))
                wk_sb = pj.tile([128, NDT, D], BF16)
                xk_sb = pj.tile([128, NDT, S], BF16)
                nc.sync.dma_start(out=wk_sb[:], in_=wkt.rearrange("(t p) o -> p t o", p=128))
                nc.sync.dma_start(out=xk_sb[:], in_=xkt.rearrange("(t p) j -> p t j", p=128))
                wv_sb = pj.tile([128, NDT, D], BF16)
                xv_sb = pj.tile([128, NDT, S], BF16)
                nc.sync.dma_start(out=wv_sb[:], in_=wvt.rearrange("(t p) o -> p t o", p=128))
                nc.sync.dma_start(out=xv_sb[:], in_=xvt.rearrange("(t p) j -> p t j", p=128))

                for m in range(NDT):
                    ps = pj_ps.tile([128, 512], F32, tag="pj")
                    for t in range(NDT):
                        nc.tensor.matmul(ps[:], wq_sb[:, t, 128 * m:128 * m + 128],
                                         xq_sb[:, t, :],
                                         start=(t == 0), stop=(t == NDT - 1))
                    nc.scalar.activation(qT[:, m, :], ps[:], ACT.Identity,
                                         bias=bq_col[:, m:m + 1])

                # qe -> dqe band steps (only needs qT)
                for qt in range(NQT):
                    qe_psum = qe_ps.tile([128, H * NUM_EMB], F32, tag="qe")
                    for m in range(NDT):
                        nc.tensor.matmul(qe_psum[:, 20 * m:20 * m + 20],
                                         qT[:, m, 128 * qt:128 * qt + 128],
                                         embT_blk[:],
                                         start=True, stop=True)
                    qe_sb = prep.tile([128, H, NUM_EMB], F32, tag="qe_sb")
                    nc.scalar.copy(qe_sb[:], qe_psum[:].rearrange("p (h e) -> p h e", e=NUM_EMB))
                    nc.vector.tensor_tensor(out=dqe[:, qt, :, :],
                                            in0=qe_sb[:, :, 1:1 + n_e],
                                            in1=qe_sb[:, :, 0:n_e], op=ALU.subtract)

                for m in range(NDT):
                    for hf in range(2):
                        ps = pj_ps.tile([128, 512], F32, tag="pj")
                        for t in range(NDT):
                            nc.tensor.matmul(ps[:], wk_sb[:, t, 128 * m:128 * m + 128],
                                             xk_sb[:, t, 512 * hf:512 * hf + 512],
                                             start=(t == 0), stop=(t == NDT - 1))
                        nc.scalar.activation(kT[:, m, 512 * hf:512 * hf + 512], ps[:],
                                             ACT.Identity, bias=bk_col[:, m:m + 1])

                for m in range(NKT):
                    for hf in range(2):
                        ps = pj_ps.tile([128, 384], F32, tag="pj")
                        for t in range(NDT):
                            nc.tensor.matmul(ps[:], xv_sb[:, t, 128 * m:128 * m + 128],
                                             wv_sb[:, t, 384 * hf:384 * hf + 384],
                                             start=(t == 0), stop=(t == NDT - 1))
                        # scatter 6 head slices [128, 6, 64] -> v_sb[:, m, 6hf:6hf+6, 0:64]
                        nc.scalar.copy(
                            v_sb[:, m, 6 * hf:6 * hf + 6, 0:DK],
                            ps[:].rearrange("p (h d) -> p h d", d=DK))

            mprep.__exit__(None, None, None)

            # ---- attention: software-pipelined over (qt, h) ----
            with tc.tile_pool(name="att", bufs=2) as att, \
                 tc.tile_pool(name="osb", bufs=2) as osb, \
                 tc.tile_pool(name="qk_ps", bufs=2, space="PSUM") as qk_ps, \
                 tc.tile_pool(name="pt_ps", bufs=2, space="PSUM") as pt_ps, \
                 tc.tile_pool(name="av_ps", bufs=2, space="PSUM") as av_ps:

                prev = None          # (qt, h, p_sb, o_tile)
                pend = None          # (qt, h, av, o_tile) awaiting normalize
                o_tile = None

                def finish_pe(prev):
                    """transpose P (PE), copy to SBUF (Act), AV matmuls (PE)."""
                    qt_p, h_p, p_sb, o_t = prev
                    ptp = pt_ps.tile([128, NKT, 128], BF16, tag="ptp")
                    for c in range(NKT):
                        nc.tensor.transpose(ptp[:, c, :], p_sb[:, 128 * c:128 * c + 128], ident[:])
                    pT = att.tile([128, NKT, 128], BF16, tag="pT")
                    nc.scalar.copy(pT[:], ptp[:])
                    av = av_ps.tile([128, DK + 1], F32, tag="av")
                    for c in range(NKT):
                        nc.tensor.matmul(av[:], pT[:, c, :], v_sb[:, c, h_p, :],
                                         start=(c == 0), stop=(c == NKT - 1))
                    return (qt_p, h_p, av, o_t)

                def finish_post(pend):
                    """normalize by the ones-column row sum, add bv, store."""
                    qt_p, h_p, av, o_t = pend
                    recip = att.tile([128, 1], F32, tag="recip")
                    nc.vector.reciprocal(recip[:], av[:, DK:DK + 1])
                    nc.vector.scalar_tensor_tensor(
                        out=o_t[:, h_p, :], in0=av[:, 0:DK], scalar=recip[:],
                        in1=bv_b[:, DK * h_p:DK * h_p + DK], op0=ALU.mult, op1=ALU.add)
                    if h_p == H - 1:
                        nc.sync.dma_start(
                            out=out[128 * qt_p:128 * qt_p + 128, :],
                            in_=o_t[:].rearrange("p h d -> p (h d)"))

                # n_act band tiles per head are produced on the scalar engine,
                # PREFETCHED one head ahead so the in-order Act queue never
                # stalls the injects (issuing them same-head cost ~13us in v5).
                n_act = 2

                def make_tt(qt_n, h_n):
                    tt_n = att.tile([128, n_e, S], BF16, tag="tt", bufs=3)
                    for e in range(n_e - n_act, n_e):
                        nc.scalar.activation(tt_n[:, e, :], masks_all[:, qt_n, e, :],
                                             ACT.Identity,
                                             scale=dqe[:, qt_n, h_n, e:e + 1])
                    return tt_n

                pairs = [(qt, h) for qt in range(NQT) for h in range(H)]
                tt = None
                for idx, (qt, h) in enumerate(pairs):
                    if h == 0:
                        o_tile = osb.tile([128, H, DK], F32, tag="o")
                    off = (64 * h) % 128
                    if tt is None:
                        tt = make_tt(qt, h)
                    # prefetch next head's Act band tiles
                    tt_nxt = make_tt(*pairs[idx + 1]) if idx + 1 < len(pairs) else None
                    # PE leads with prev head's transpose+AV (ready work)
                    if prev is not None:
                        pend = finish_pe(prev)
                    # --- bias band tiles: t_e = mask_e * dqe_e, bf16 4x ---
                    for e in range(n_e - n_act):
                        nc.vector.tensor_scalar(
                            out=tt[:, e, :], in0=masks_all[:, qt, e, :],
                            scalar1=dqe[:, qt, h, e:e + 1], scalar2=None,
                            op0=ALU.mult)
                    # --- small DVE tree over the first n_dve tiles ---
                    # (GpSimd adds were tried here: they contend with DVE
                    # for SBUF ports and slowed every DVE op ~50%.)
                    r = tt[:, 0, :]
                    for d in range(1, n_dve):
                        racc = att.tile([128, S], BF16, tag=f"racc{d % 2}")
                        nc.vector.tensor_tensor(out=racc[:], in0=r, in1=tt[:, d, :], op=ALU.add)
                        r = racc[:]
                    inj = [tt[:, e, :] for e in range(n_dve, n_e)] + [r]
                    # --- qk + injected bias accumulation in PSUM ---
                    qk = qk_ps.tile([128, S], F32, tag="qk")
                    for hf in range(2):
                        sl = slice(512 * hf, 512 * hf + 512)
                        nc.tensor.matmul(qk[:, sl],
                                         qT[off:off + 64, h // 2, 128 * qt:128 * qt + 128],
                                         kT[off:off + 64, h // 2, sl],
                                         start=True, stop=False)
                        for ii, tsl in enumerate(inj):
                            nc.tensor.matmul(qk[:, sl], ident[:], tsl[:, sl],
                                             start=False, stop=(ii == len(inj) - 1))
                    # --- P = exp(logits/8) ---
                    p_sb = att.tile([128, S], BF16, tag="p")
                    nc.scalar.activation(p_sb[:], qk[:], ACT.Exp, scale=0.125)
                    if pend is not None:
                        finish_post(pend)
                        pend = None
                    prev = (qt, h, p_sb, o_tile)
                    tt = tt_nxt
                # drain
                finish_post(finish_pe(prev))
    nc.compile()
    return nc


_NC_CACHE = {}


def _get_nc(n_e=None):
    if n_e is None:
        n_e = _NC_CACHE.get("last", 8)
    if n_e not in _NC_CACHE:
        _NC_CACHE[n_e] = build_nc(n_e=n_e)
    _NC_CACHE["last"] = n_e
    return _NC_CACHE[n_e]


def _make_in_maps(inputs):
    query = np.asarray(inputs["query"], dtype=np.float32)
    key = np.asarray(inputs["key"], dtype=np.float32)
    value = np.asarray(inputs["value"], dtype=np.float32)
    tp = np.asarray(inputs["tile_positions"], dtype=np.float32)
    Wq = np.asarray(inputs["Wq"], dtype=np.float32)
    Wk = np.asarray(inputs["Wk"], dtype=np.float32)
    Wv = np.asarray(inputs["Wv"], dtype=np.float32)
    bq = np.asarray(inputs["bq"], dtype=np.float32)
    bk = np.asarray(inputs["bk"], dtype=np.float32)
    bv = np.asarray(inputs["bv"], dtype=np.float32)
    emb = np.asarray(inputs["emb_k"], dtype=np.float32)

    wqt = np.ascontiguousarray(Wq.T.astype(BF))
    wkt = np.ascontiguousarray(Wk.T.astype(BF))
    wvt = np.ascontiguousarray(Wv.T.astype(BF))
    embt = np.ascontiguousarray(emb.T.astype(BF))

    in_maps = []
    for c in range(NCORES):
        b, qh = c // 2, c % 2
        sl = slice(qh * SQ, (qh + 1) * SQ)
        in_maps.append({
            "xqt": np.ascontiguousarray(query[b, sl].T.astype(BF)),
            "xkt": np.ascontiguousarray(key[b].T.astype(BF)),
            "xvt": np.ascontiguousarray(value[b].T.astype(BF)),
            "wqt": wqt, "wkt": wkt, "wvt": wvt, "embt": embt,
            "bq": bq, "bk": bk, "bv": bv,
            "pkx": np.ascontiguousarray(tp[b, :, 0]),
            "pky": np.ascontiguousarray(tp[b, :, 1]),
            "pqx": np.ascontiguousarray(tp[b, sl, 0]),
            "pqy": np.ascontiguousarray(tp[b, sl, 1]),
        })
    return in_maps


def _active_bands(tp):
    """Highest band index that actually occurs for these positions."""
    mx = 0.0
    for b in range(tp.shape[0]):
        p = tp[b]
        d2 = ((p[:, None, :] - p[None, :, :]) ** 2).sum(-1)
        mx = max(mx, float(d2.max()))
    max_idx = int(np.floor(9.0 * np.sqrt(mx) / MAX_DIST + 0.5))
    return max(1, min(max_idx, NUM_EMB - 1))


def kernel(query, key, value, tile_positions, Wq, bq, Wk, bk, Wv, bv, emb_k):
    inputs = {"query": query, "key": key, "value": value,
              "tile_positions": tile_positions,
              "Wq": Wq, "bq": bq, "Wk": Wk, "bk": bk, "Wv": Wv, "bv": bv,
              "emb_k": emb_k}
    tp = np.asarray(tile_positions, dtype=np.float32)
    n_e = _active_bands(tp)
    nc = _get_nc(n_e)
    in_maps = _make_in_maps(inputs)
    res = run_bass_kernel_spmd(nc, in_maps, core_ids=list(range(NCORES)))
    out = np.empty((B, S, D), np.float32)
    for c in range(NCORES):
        b, qh = c // 2, c % 2
        out[b, qh * SQ:(qh + 1) * SQ] = res.results[c]["out"]
    return out
